# revision 1
# baseline (speedup 1.0000x reference)
"""Causal self-attention (B=2, S=2048, E=1024, H=16) on 8 TRN2 NeuronCores.

Sharding: core c = 4*b + g handles batch b and head-group g (4 heads,
256 E-columns). Each core computes q/k/v projections for its head slice,
causal flash-style attention for its 4 heads, and a partial output
projection y_c = ctx_g @ Wo[rows_g].  Host sums the 4 partials per batch
and adds bo.

Device dataflow (per core), f32r on all matmul paths:
  xT [E,S] (host-pretransposed) -> qT/kT [2x128, S] (head-major: head h in
  tile h//2, partitions (h%2)*64..) and v1 [S, 4x(64+1)] (natural layout +
  ones column -> softmax denominator rides the attention matmul).
  Per q-chunk (512) x head-pair: one [128,1024] PSUM tile holds both
  heads' scoresT for a k-tile (K=64 matmuls at base partitions 0/64 run
  concurrently in separate PE row groups), one ACT exp covers both heads,
  causal masking multiplies a 0/1 triangle into the diagonal 128-block
  (gpsimd), ctxT[65,512] += v1-tile.T @ expT (K=128; row 64 = denominator).
  Normalization: DVE reciprocal of the PSUM denominator row -> gpsimd
  partition_broadcast -> DVE multiply. Output projection uses ctxT as lhsT.
  Causal trimming: for diagonal k-tile t' only q-columns >= 128*t' are
  computed (scores matmul, exp, ctx matmul all restricted).
  Emission interleaves projection chunks with attention q-chunks so ACT
  (exp) work overlaps projection-phase PE work.
"""

import os

import numpy as np

os.environ.setdefault("NEURON_RT_RESET_CORES", "1")

B, S, E, H, D = 2, 2048, 1024, 16, 64
NCORES = 8
EC = 256          # E-columns per core (4 heads x 64)
QC = 512          # q-chunk width
NQC = S // QC     # 4
NKT = S // 128    # 16 k-tiles
NE = E // 128     # 8 contraction chunks

_CACHE = {}


def _build_nc(cfg=None):
    cfg = cfg or {}
    MM_BUFS = cfg.get("mm", 2)
    CX_BUFS = cfg.get("cx", 3)
    PY_BUFS = cfg.get("py", 1)
    EXP_BUFS = cfg.get("exp", 4)
    CTX_BUFS = cfg.get("ctx", 4)
    import concourse.mybir as mybir
    import concourse.tile as tile
    import concourse.bass as bass
    from concourse import bacc

    F32 = mybir.dt.float32
    F32R = mybir.dt.float32r
    EXP = mybir.ActivationFunctionType.Exp

    nc = bacc.Bacc("TRN2", target_bir_lowering=False, debug=False)

    xT = nc.dram_tensor("xT", [E, S], F32R, kind="ExternalInput")
    wq = nc.dram_tensor("wq", [E, EC], F32R, kind="ExternalInput")
    wk = nc.dram_tensor("wk", [E, EC], F32R, kind="ExternalInput")
    wv = nc.dram_tensor("wv", [E, EC], F32R, kind="ExternalInput")
    wo = nc.dram_tensor("wo", [EC, E], F32R, kind="ExternalInput")
    bq = nc.dram_tensor("bq", [2, 128, 1], F32, kind="ExternalInput")
    bk = nc.dram_tensor("bk", [2, 128, 1], F32, kind="ExternalInput")
    bv = nc.dram_tensor("bv", [1, EC], F32, kind="ExternalInput")
    msk = nc.dram_tensor("msk", [128, 128], F32R, kind="ExternalInput")
    ones = nc.dram_tensor("ones", [1, 64], F32R, kind="ExternalInput")

    y = nc.dram_tensor("y", [S, E], F32, kind="ExternalOutput")

    with tile.TileContext(nc) as tc:
        with (
            tc.tile_pool(name="weights", bufs=1) as wpool,
            tc.tile_pool(name="xtp", bufs=1) as xtp,
            tc.tile_pool(name="qkv", bufs=1) as qkv,
            tc.tile_pool(name="expp", bufs=EXP_BUFS) as expp,
            tc.tile_pool(name="ctxn", bufs=CTX_BUFS) as ctxp,
            tc.tile_pool(name="odd", bufs=2) as oddp,
            tc.tile_pool(name="yp", bufs=4) as yp,
            tc.tile_pool(name="rows", bufs=3) as rows,
            tc.tile_pool(name="smalls", bufs=1) as smalls,
            tc.tile_pool(name="mm", bufs=MM_BUFS, space="PSUM") as mmp,
            tc.tile_pool(name="cx", bufs=CX_BUFS, space="PSUM") as cxp,
            tc.tile_pool(name="pyp", bufs=PY_BUFS, space="PSUM") as pyp,
        ):
            # ---- small constants (SWDGE/Pool queue; SP stays free) ----
            tbq = smalls.tile([128, 2], F32, tag="bq")
            tbk = smalls.tile([128, 2], F32, tag="bk")
            tbv = smalls.tile([128, EC], F32, tag="bv")
            tmsk = smalls.tile([128, 128], F32R, tag="msk")
            tones = smalls.tile([1, 64], F32R, tag="ones")

            for r in range(2):
                nc.gpsimd.dma_start(tbq[:, r:r + 1], bq[r])
                nc.gpsimd.dma_start(tbk[:, r:r + 1], bk[r])
            bvap = bv[0, :]
            bv_b = bass.AP(tensor=bvap.tensor, offset=bvap.offset,
                           ap=[[0, 128]] + list(bvap.ap))
            nc.gpsimd.dma_start(tbv[:], bv_b)
            nc.gpsimd.dma_start(tmsk[:], msk[:])
            nc.gpsimd.dma_start(tones[:], ones[:])

            # ---- bulk inputs: single DMA per weight tensor ----
            twq = wpool.tile([128, NE, EC], F32R, tag="wq")
            twk = wpool.tile([128, NE, EC], F32R, tag="wk")
            twv = wpool.tile([128, NE, EC], F32R, tag="wv")
            two = wpool.tile([128, 2, E], F32R, tag="wo")

            def chunked(dram, nch, width):
                # [nch*128, width] DRAM -> [128, nch, width] SBUF view
                a = dram[:]
                return bass.AP(tensor=a.tensor, offset=a.offset,
                               ap=[[width, 128], [128 * width, nch], [1, width]])

            txt = [xtp.tile([128, S], F32R, tag=f"xt{e}", name=f"xt{e}")
                   for e in range(NE)]
            if cfg.get("ord", "B") == "B":
                nsp = cfg.get("nsplit", 4)
                def ldx(e):
                    w = S // nsp
                    for i in range(nsp):
                        nc.sync.dma_start(
                            txt[e][:, i * w:(i + 1) * w],
                            xT[e * 128:(e + 1) * 128, i * w:(i + 1) * w])
                ldx(0)
                nc.sync.dma_start(twq[:], chunked(wq, NE, EC))
                nc.sync.dma_start(twk[:], chunked(wk, NE, EC))
                nc.sync.dma_start(twv[:], chunked(wv, NE, EC))
                for e in range(1, NE):
                    ldx(e)
                nc.sync.dma_start(two[:], chunked(wo, 2, E))
            else:
                nc.sync.dma_start(txt[0][:], xT[0:128, :])
                nc.sync.dma_start(twq[:], chunked(wq, NE, EC))
                nc.sync.dma_start(txt[1][:], xT[128:256, :])
                nc.sync.dma_start(twk[:], chunked(wk, NE, EC))
                nc.sync.dma_start(txt[2][:], xT[256:384, :])
                nc.sync.dma_start(twv[:], chunked(wv, NE, EC))
                for e in range(3, NE):
                    nc.sync.dma_start(txt[e][:], xT[e * 128:(e + 1) * 128, :])
                nc.sync.dma_start(two[:], chunked(wo, 2, E))

            # ---- persistent activation tiles ----
            tq = [qkv.tile([128, S], F32R, tag=f"q{r}", name=f"q{r}")
                  for r in range(2)]
            tk = [qkv.tile([128, S], F32R, tag=f"k{r}", name=f"k{r}")
                  for r in range(2)]
            # v1: [128, s-tile, head, 65]; col 64 of each head block = 1.0
            tv = qkv.tile([128, NKT, 4, 65], F32R, tag="v")

            onesap = ones[0, 0:1]
            ones_v = bass.AP(tensor=onesap.tensor, offset=onesap.offset,
                             ap=[[0, 128], [0, NKT * 4], [0, 1]])
            nc.gpsimd.dma_start(tv[:, :, :, 64:65], ones_v)

            # broadcast tri-mask [128,128] over the two head-halves
            def mask_b(n):
                m = tmsk[:]
                return bass.AP(tensor=m.tensor, offset=m.offset,
                               ap=[list(m.ap[0]), [0, 2], [1, n]])

            def proj_wave(scn):
                """One wave: q/k for s-chunk scn (4 units) + v for the 4
                s-tiles of chunk scn, spread over all PSUM pools so 8
                accumulations progress while xT chunks stream in.
                (PSUM accumulation groups are bank-granular, so one unit
                per bank.)"""
                sc = slice(scn * QC, (scn + 1) * QC)
                ptiles = [mmp.tile([128, 2 * QC], F32, tag="mm",
                                   name=f"pw{scn}_{i}") for i in range(2)]
                qk_units = []
                for r in range(2):
                    qk_units.append((ptiles[r][:, 0:QC], twq, r))
                    qk_units.append((ptiles[r][:, QC:2 * QC], twk, r))
                v_ps = [cxp.tile([128, QC], F32, tag="cx", name=f"pv{st}")
                        if i < 3 else
                        pyp.tile([128, QC], F32, tag="py", name=f"pv{st}")
                        for i, st in enumerate(range(4 * scn, 4 * scn + 4))]
                for e in range(NE):
                    for out_ap, w, r in qk_units:
                        nc.tensor.matmul(
                            out_ap, w[:, e, r * 128:(r + 1) * 128],
                            txt[e][:, sc],
                            start=(e == 0), stop=(e == NE - 1))
                    for i, st in enumerate(range(4 * scn, 4 * scn + 4)):
                        nc.tensor.matmul(
                            v_ps[i][:, 0:EC],
                            txt[e][:, st * 128:(st + 1) * 128], twv[:, e, :],
                            start=(e == 0), stop=(e == NE - 1))
                for r in range(2):
                    nc.vector.tensor_scalar_add(
                        tq[r][:, sc], ptiles[r][:, 0:QC], tbq[:, r:r + 1])
                    nc.vector.tensor_scalar_add(
                        tk[r][:, sc], ptiles[r][:, QC:2 * QC], tbk[:, r:r + 1])
                for i, st in enumerate(range(4 * scn, 4 * scn + 4)):
                    nc.vector.tensor_add(
                        tv[:, st, :, 0:64],
                        v_ps[i][:, 0:EC].rearrange("p (h d) -> p h d", h=4),
                        tbv[:].rearrange("p (h d) -> p h d", h=4))

            def attention(qc):
                n_kt = 4 * (qc + 1)
                ctx_sb = [None, None]
                for hp in range(2):
                    ctx_sbuf = ctxp.tile([128, QC], F32R, tag="ctxn",
                                         name=f"ctx{qc}_{hp}")
                    ctx_sb[hp] = ctx_sbuf
                    pctx = [cxp.tile([65, QC], F32, tag="cx",
                                     name=f"cx{qc}_{hp}_{i}")
                            for i in range(2)]
                    for kt in range(n_kt):
                        dg = kt - 4 * qc  # >=0: diagonal tile index
                        coff = 128 * dg if dg > 0 else 0
                        ps = mmp.tile([128, 2 * QC], F32, tag="mm",
                                      name=f"ps{qc}_{hp}_{kt}")
                        te = expp.tile([128, 2 * QC], F32R, tag="exp",
                                       name=f"te{qc}_{hp}_{kt}")
                        for h2 in range(2):
                            bp = h2 * 64
                            nc.tensor.matmul(
                                ps[:, h2 * QC + coff:(h2 + 1) * QC],
                                tk[hp][bp:bp + 64, kt * 128:(kt + 1) * 128],
                                tq[hp][bp:bp + 64,
                                       qc * QC + coff:(qc + 1) * QC],
                                start=True, stop=True)
                        if coff:
                            ps3 = ps[:].rearrange("p (t n) -> p t n", t=2)
                            te3 = te[:].rearrange("p (t n) -> p t n", t=2)
                            nc.scalar.activation(
                                te3[:, :, coff:QC], ps3[:, :, coff:QC], EXP)
                        else:
                            nc.scalar.activation(te[:], ps[:], EXP)
                        if dg >= 0:
                            te3 = te[:].rearrange("p (t n) -> p t n", t=2)
                            eng = nc.vector if cfg.get("mask_dve") else nc.gpsimd
                            eng.tensor_mul(
                                te3[:, :, coff:coff + 128],
                                te3[:, :, coff:coff + 128],
                                mask_b(128))
                        for h2 in range(2):
                            h = 2 * hp + h2
                            nc.tensor.matmul(
                                pctx[h2][:, coff:QC],
                                tv[:, kt, h, :],
                                te[:, h2 * QC + coff:(h2 + 1) * QC],
                                start=(kt == 0), stop=(kt == n_kt - 1))
                    # normalization (denominator = pctx row 64); odd head
                    # first so its partition-shift DMA overlaps the even mul
                    for h2 in (1, 0):
                        rec1 = rows.tile([1, QC], F32, tag="rec1")
                        nc.vector.reciprocal(rec1[:], pctx[h2][64:65, :])
                        rec = rows.tile([64, QC], F32, tag="rec")
                        nc.gpsimd.partition_broadcast(rec[:], rec1[:])
                        if h2 == 0:
                            nc.vector.tensor_mul(
                                ctx_sbuf[0:64, :], pctx[h2][0:64, :], rec[:])
                        else:
                            tmp = oddp.tile([64, QC], F32R, tag="odd")
                            nc.vector.tensor_mul(
                                tmp[:], pctx[h2][0:64, :], rec[:])
                            nc.sync.dma_start(ctx_sbuf[64:128, :], tmp[:])
                return ctx_sb

            def out_proj(qc, ctx_sb, last=False):
                for ss in range(4):
                    s0 = qc * QC + ss * 128
                    for nn in range(2):
                        # the final q-chunk's projections also draw from the
                        # (idle by then) scores pool for deeper pipelining
                        if last and (ss * 2 + nn) % 2 == 1:
                            py = mmp.tile([128, 2 * QC], F32, tag="mm",
                                          name=f"py{qc}_{ss}_{nn}")
                        else:
                            py = pyp.tile([128, QC], F32, tag="py",
                                          name=f"py{qc}_{ss}_{nn}")
                        for hp in range(2):
                            nc.tensor.matmul(
                                py[:, 0:QC],
                                ctx_sb[hp][:, ss * 128:(ss + 1) * 128],
                                two[:, hp, nn * QC:(nn + 1) * QC],
                                start=(hp == 0), stop=(hp == 1))
                        ysb = yp.tile([128, QC], F32, tag="y",
                                      name=f"y{qc}_{ss}_{nn}")
                        if cfg.get("ycopy", "dve") == "act":
                            nc.scalar.copy(ysb[:], py[:, 0:QC])
                        else:
                            nc.vector.tensor_copy(ysb[:], py[:, 0:QC])
                        nc.sync.dma_start(
                            y[s0:s0 + 128, nn * QC:(nn + 1) * QC], ysb[:])

            # interleave projection blocks with attention q-chunks; process
            # the longest q-chunk right after projections and end on the
            # shortest to minimize the kernel tail
            if cfg.get("inner"):
                for blk in range(NQC):
                    proj_wave(blk)
                    if blk >= 1:
                        out_proj(blk - 1, attention(blk - 1))
                out_proj(NQC - 1, attention(NQC - 1), last=True)
            else:
                qc_order = cfg.get("qc_order", [0, 1, 2, 3])
                for blk in range(NQC):
                    proj_wave(blk)
                    if blk == 1:
                        out_proj(0, attention(0))
                for qc in qc_order[1:]:
                    out_proj(qc, attention(qc), last=(qc == qc_order[-1]))

    nc.compile()
    return nc


def _get_nc():
    if "nc" not in _CACHE:
        _CACHE["nc"] = _build_nc()
    return _CACHE["nc"]


def make_mask():
    kl = np.arange(128)[:, None]
    ql = np.arange(128)[None, :]
    return (ql >= kl).astype(np.float32)


def shard_inputs(x, Wq, bq, Wk, bk, Wv, bv, Wo, bo):
    """Build the 8 per-core input maps (host-side sharding)."""
    x = np.asarray(x, dtype=np.float32)
    scale = np.float32(1.0 / np.sqrt(D))
    mask = make_mask()
    ones = np.ones((1, 64), np.float32)
    in_maps = []
    xTb = [np.ascontiguousarray(np.asarray(x[b]).T) for b in range(B)]
    for c in range(NCORES):
        b, g = divmod(c, 4)
        cs = slice(g * EC, (g + 1) * EC)
        in_maps.append({
            "xT": xTb[b],
            "wq": np.ascontiguousarray(np.asarray(Wq[:, cs]) * scale),
            "wk": np.ascontiguousarray(np.asarray(Wk[:, cs])),
            "wv": np.ascontiguousarray(np.asarray(Wv[:, cs])),
            "wo": np.ascontiguousarray(np.asarray(Wo[cs, :])),
            "bq": (np.asarray(bq[cs]) * scale).reshape(2, 128, 1).astype(np.float32),
            "bk": np.asarray(bk[cs]).reshape(2, 128, 1).astype(np.float32),
            "bv": np.asarray(bv[cs]).reshape(1, EC).astype(np.float32),
            "msk": mask,
            "ones": ones,
        })
    return in_maps


def combine_outputs(results, bo):
    y = np.zeros((B, S, E), np.float32)
    for c in range(NCORES):
        b = c // 4
        y[b] += results[c]["y"]
    y += np.asarray(bo, dtype=np.float32)[None, None, :]
    return y


def kernel(x, Wq, bq, Wk, bk, Wv, bv, Wo, bo):
    from concourse.bass_utils import run_bass_kernel_spmd

    nc = _get_nc()
    in_maps = shard_inputs(x, Wq, bq, Wk, bk, Wv, bv, Wo, bo)
    try:
        res = run_bass_kernel_spmd(nc, in_maps, core_ids=list(range(NCORES)))
    except Exception:
        # transient device errors (e.g. a wedged core) usually clear on retry
        res = run_bass_kernel_spmd(nc, in_maps, core_ids=list(range(NCORES)))
    return combine_outputs(res.results, bo)



# revision 2
# speedup vs baseline: 1.1816x; 1.1816x over previous
"""Causal self-attention (B=2, S=2048, E=1024, H=16) on 8 TRN2 NeuronCores.

Sharding: core c = 4*b + g handles batch b and head-group g (4 heads,
256 E-columns). Each core computes q/k/v projections for its head slice,
causal flash-style attention for its 4 heads, and a partial output
projection y_c = ctx_g @ Wo[rows_g].  Host sums the 4 partials per batch
and adds bo.

All matmul operands are bf16 (inputs converted on host); PSUM accumulation
stays f32.  xT streams in s-chunk-major order interleaved with split
weight loads so the first projection wave starts as soon as its first
weight half + x piece land, and later waves never starve on DMA.

Device dataflow (per core):
  xT [E,S] (host-pretransposed, bf16) -> qT/kT [2x128, S] (head-major:
  head h in tile h//2, partitions (h%2)*64..) and v1 [S, 4x(64+1)]
  (natural layout + ones column -> softmax denominator rides the
  attention matmul).
  Per q-chunk (512) x head-pair: one [128,1024] PSUM tile holds both
  heads' scoresT for a k-tile (K=64 matmuls at base partitions 0/64 run
  concurrently in separate PE row groups), one ACT exp covers both heads,
  causal masking multiplies a 0/1 triangle into the diagonal 128-block
  (gpsimd), ctxT[65,512] += v1-tile.T @ expT (K=128; row 64 = denominator).
  Normalization: DVE reciprocal of the PSUM denominator row -> gpsimd
  partition_broadcast -> DVE multiply. Output projection uses ctxT as lhsT.
  Causal trimming: for diagonal k-tile t' only q-columns >= 128*t' are
  computed (scores matmul, exp, ctx matmul all restricted).
"""

import os

import numpy as np
import ml_dtypes

os.environ.setdefault("NEURON_RT_RESET_CORES", "1")

B, S, E, H, D = 2, 2048, 1024, 16, 64
NCORES = 8
EC = 256          # E-columns per core (4 heads x 64)
QC = 512          # q-chunk width
NQC = S // QC     # 4
NKT = S // 128    # 16 k-tiles
NE = E // 128     # 8 contraction chunks

BF = ml_dtypes.bfloat16

_CACHE = {}


def _build_nc(cfg=None):
    cfg = cfg or {}
    MM_BUFS = cfg.get("mm", 2)
    CX_BUFS = cfg.get("cx", 3)
    PY_BUFS = cfg.get("py", 1)
    EXP_BUFS = cfg.get("exp", 4)
    CTX_BUFS = cfg.get("ctx", 4)
    import concourse.mybir as mybir
    import concourse.tile as tile
    import concourse.bass as bass
    from concourse import bacc

    F32 = mybir.dt.float32
    BF16 = mybir.dt.bfloat16
    EXP = mybir.ActivationFunctionType.Exp

    nc = bacc.Bacc("TRN2", target_bir_lowering=False, debug=False)

    xT = nc.dram_tensor("xT", [E, S], BF16, kind="ExternalInput")
    wq = nc.dram_tensor("wq", [E, EC], BF16, kind="ExternalInput")
    wk = nc.dram_tensor("wk", [E, EC], BF16, kind="ExternalInput")
    wv = nc.dram_tensor("wv", [E, EC], BF16, kind="ExternalInput")
    wo = nc.dram_tensor("wo", [EC, E], BF16, kind="ExternalInput")
    bq = nc.dram_tensor("bq", [2, 128, 1], F32, kind="ExternalInput")
    bk = nc.dram_tensor("bk", [2, 128, 1], F32, kind="ExternalInput")
    bv = nc.dram_tensor("bv", [1, EC], F32, kind="ExternalInput")
    msk = nc.dram_tensor("msk", [128, 128], BF16, kind="ExternalInput")
    ones = nc.dram_tensor("ones", [1, 64], BF16, kind="ExternalInput")

    y = nc.dram_tensor("y", [S, E], BF16, kind="ExternalOutput")

    with tile.TileContext(nc) as tc:
        with (
            tc.tile_pool(name="weights", bufs=1) as wpool,
            tc.tile_pool(name="xtp", bufs=1) as xtp,
            tc.tile_pool(name="qkv", bufs=1) as qkv,
            tc.tile_pool(name="expp", bufs=EXP_BUFS) as expp,
            tc.tile_pool(name="ctxn", bufs=CTX_BUFS) as ctxp,
            tc.tile_pool(name="odd", bufs=2) as oddp,
            tc.tile_pool(name="yp", bufs=4) as yp,
            tc.tile_pool(name="rows", bufs=3) as rows,
            tc.tile_pool(name="smalls", bufs=1) as smalls,
            tc.tile_pool(name="mm", bufs=MM_BUFS, space="PSUM") as mmp,
            tc.tile_pool(name="cx", bufs=CX_BUFS, space="PSUM") as cxp,
            tc.tile_pool(name="pyp", bufs=PY_BUFS, space="PSUM") as pyp,
        ):
            # ---- small constants (SWDGE/Pool queue; SP stays free) ----
            tbq = smalls.tile([128, 2], F32, tag="bq")
            tbk = smalls.tile([128, 2], F32, tag="bk")
            tbv = smalls.tile([128, EC], F32, tag="bv")
            tmsk = smalls.tile([128, 128], BF16, tag="msk")
            tones = smalls.tile([1, 64], BF16, tag="ones")

            for r in range(2):
                nc.gpsimd.dma_start(tbq[:, r:r + 1], bq[r])
                nc.gpsimd.dma_start(tbk[:, r:r + 1], bk[r])
            bvap = bv[0, :]
            bv_b = bass.AP(tensor=bvap.tensor, offset=bvap.offset,
                           ap=[[0, 128]] + list(bvap.ap))
            nc.gpsimd.dma_start(tbv[:], bv_b)
            nc.gpsimd.dma_start(tmsk[:], msk[:])
            nc.gpsimd.dma_start(tones[:], ones[:])

            # ---- bulk inputs: s-chunk-major streaming ----
            twq = wpool.tile([128, NE, EC], BF16, tag="wq")
            twk = wpool.tile([128, NE, EC], BF16, tag="wk")
            twv = wpool.tile([128, NE, EC], BF16, tag="wv")
            two = wpool.tile([128, 2, E], BF16, tag="wo")

            def chunked_half(dram, width, h, nch=4):
                # rows h*nch*128 ... of [8*128, width] DRAM
                # -> [128, nch, width] SBUF view
                a = dram[:]
                return bass.AP(tensor=a.tensor,
                               offset=a.offset + h * nch * 128 * width,
                               ap=[[width, 128], [128 * width, nch],
                                   [1, width]])

            def chunked(dram, nch, width):
                a = dram[:]
                return bass.AP(tensor=a.tensor, offset=a.offset,
                               ap=[[width, 128], [128 * width, nch],
                                   [1, width]])

            txt = [xtp.tile([128, S], BF16, tag=f"xt{e}", name=f"xt{e}")
                   for e in range(NE)]

            def ldx(e, sc):
                nc.sync.dma_start(
                    txt[e][:, sc * QC:(sc + 1) * QC],
                    xT[e * 128:(e + 1) * 128, sc * QC:(sc + 1) * QC])

            # wave-0 critical path: wqA -> x(0,0) -> wkA, wvA -> rest
            nc.sync.dma_start(twq[:, 0:4, :], chunked_half(wq, EC, 0))
            ldx(0, 0)
            nc.sync.dma_start(twk[:, 0:4, :], chunked_half(wk, EC, 0))
            nc.sync.dma_start(twv[:, 0:4, :], chunked_half(wv, EC, 0))
            ldx(1, 0)
            ldx(2, 0)
            ldx(3, 0)
            nc.sync.dma_start(twq[:, 4:8, :], chunked_half(wq, EC, 1))
            nc.sync.dma_start(twk[:, 4:8, :], chunked_half(wk, EC, 1))
            nc.sync.dma_start(twv[:, 4:8, :], chunked_half(wv, EC, 1))
            for e in range(4, NE):
                ldx(e, 0)
            for e in range(NE):
                ldx(e, 1)
            nc.sync.dma_start(two[:], chunked(wo, 2, E))
            for sc in range(2, NQC):
                for e in range(NE):
                    ldx(e, sc)

            # ---- persistent activation tiles ----
            tq = [qkv.tile([128, S], BF16, tag=f"q{r}", name=f"q{r}")
                  for r in range(2)]
            tk = [qkv.tile([128, S], BF16, tag=f"k{r}", name=f"k{r}")
                  for r in range(2)]
            # v1: [128, s-tile, head, 65]; col 64 of each head block = 1.0
            tv = qkv.tile([128, NKT, 4, 65], BF16, tag="v")

            onesap = ones[0, 0:1]
            ones_v = bass.AP(tensor=onesap.tensor, offset=onesap.offset,
                             ap=[[0, 128], [0, NKT * 4], [0, 1]])
            nc.gpsimd.dma_start(tv[:, :, :, 64:65], ones_v)

            # broadcast tri-mask [128,128] over the two head-halves
            def mask_b(n):
                m = tmsk[:]
                return bass.AP(tensor=m.tensor, offset=m.offset,
                               ap=[list(m.ap[0]), [0, 2], [1, n]])

            def proj_wave(scn):
                """One wave: q/k for s-chunk scn (4 units) + v for the 4
                s-tiles of chunk scn, spread over all PSUM pools so 8
                accumulations progress while xT chunks stream in.
                (PSUM accumulation groups are bank-granular, so one unit
                per bank.)"""
                sc = slice(scn * QC, (scn + 1) * QC)
                ptiles = [mmp.tile([128, 2 * QC], F32, tag="mm",
                                   name=f"pw{scn}_{i}") for i in range(2)]
                qk_units = []
                for r in range(2):
                    qk_units.append((ptiles[r][:, 0:QC], twq, r))
                    qk_units.append((ptiles[r][:, QC:2 * QC], twk, r))
                v_ps = [cxp.tile([128, QC], F32, tag="cx", name=f"pv{st}")
                        if i < 3 else
                        pyp.tile([128, QC], F32, tag="py", name=f"pv{st}")
                        for i, st in enumerate(range(4 * scn, 4 * scn + 4))]
                for e in range(NE):
                    for out_ap, w, r in qk_units:
                        nc.tensor.matmul(
                            out_ap, w[:, e, r * 128:(r + 1) * 128],
                            txt[e][:, sc],
                            start=(e == 0), stop=(e == NE - 1))
                    for i, st in enumerate(range(4 * scn, 4 * scn + 4)):
                        nc.tensor.matmul(
                            v_ps[i][:, 0:EC],
                            txt[e][:, st * 128:(st + 1) * 128], twv[:, e, :],
                            start=(e == 0), stop=(e == NE - 1))
                for r in range(2):
                    nc.vector.tensor_scalar_add(
                        tq[r][:, sc], ptiles[r][:, 0:QC], tbq[:, r:r + 1])
                    nc.vector.tensor_scalar_add(
                        tk[r][:, sc], ptiles[r][:, QC:2 * QC], tbk[:, r:r + 1])
                for i, st in enumerate(range(4 * scn, 4 * scn + 4)):
                    nc.vector.tensor_add(
                        tv[:, st, :, 0:64],
                        v_ps[i][:, 0:EC].rearrange("p (h d) -> p h d", h=4),
                        tbv[:].rearrange("p (h d) -> p h d", h=4))

            def attention(qc):
                n_kt = 4 * (qc + 1)
                ctx_sb = [None, None]
                for hp in range(2):
                    ctx_sbuf = ctxp.tile([128, QC], BF16, tag="ctxn",
                                         name=f"ctx{qc}_{hp}")
                    ctx_sb[hp] = ctx_sbuf
                    pctx = [cxp.tile([65, QC], F32, tag="cx",
                                     name=f"cx{qc}_{hp}_{i}")
                            for i in range(2)]
                    for kt in range(n_kt):
                        dg = kt - 4 * qc  # >=0: diagonal tile index
                        coff = 128 * dg if dg > 0 else 0
                        ps = mmp.tile([128, 2 * QC], F32, tag="mm",
                                      name=f"ps{qc}_{hp}_{kt}")
                        te = expp.tile([128, 2 * QC], BF16, tag="exp",
                                       name=f"te{qc}_{hp}_{kt}")
                        for h2 in range(2):
                            bp = h2 * 64
                            nc.tensor.matmul(
                                ps[:, h2 * QC + coff:(h2 + 1) * QC],
                                tk[hp][bp:bp + 64, kt * 128:(kt + 1) * 128],
                                tq[hp][bp:bp + 64,
                                       qc * QC + coff:(qc + 1) * QC],
                                start=True, stop=True)
                        if coff:
                            ps3 = ps[:].rearrange("p (t n) -> p t n", t=2)
                            te3 = te[:].rearrange("p (t n) -> p t n", t=2)
                            nc.scalar.activation(
                                te3[:, :, coff:QC], ps3[:, :, coff:QC], EXP)
                        else:
                            nc.scalar.activation(te[:], ps[:], EXP)
                        if dg >= 0:
                            te3 = te[:].rearrange("p (t n) -> p t n", t=2)
                            eng = nc.vector if cfg.get("mask_dve") else nc.gpsimd
                            eng.tensor_mul(
                                te3[:, :, coff:coff + 128],
                                te3[:, :, coff:coff + 128],
                                mask_b(128))
                        for h2 in range(2):
                            h = 2 * hp + h2
                            nc.tensor.matmul(
                                pctx[h2][:, coff:QC],
                                tv[:, kt, h, :],
                                te[:, h2 * QC + coff:(h2 + 1) * QC],
                                start=(kt == 0), stop=(kt == n_kt - 1))
                    # normalization (denominator = pctx row 64); odd head
                    # first so its partition-shift DMA overlaps the even mul
                    for h2 in (1, 0):
                        rec1 = rows.tile([1, QC], F32, tag="rec1")
                        nc.vector.reciprocal(rec1[:], pctx[h2][64:65, :])
                        rec = rows.tile([64, QC], F32, tag="rec")
                        nc.gpsimd.partition_broadcast(rec[:], rec1[:])
                        if h2 == 0:
                            nc.vector.tensor_mul(
                                ctx_sbuf[0:64, :], pctx[h2][0:64, :], rec[:])
                        else:
                            tmp = oddp.tile([64, QC], BF16, tag="odd")
                            nc.vector.tensor_mul(
                                tmp[:], pctx[h2][0:64, :], rec[:])
                            nc.sync.dma_start(ctx_sbuf[64:128, :], tmp[:])
                return ctx_sb

            def out_proj(qc, ctx_sb, last=False):
                for ss in range(4):
                    s0 = qc * QC + ss * 128
                    for nn in range(2):
                        # the final q-chunk's projections also draw from the
                        # (idle by then) scores pool for deeper pipelining
                        if last and (ss * 2 + nn) % 2 == 1:
                            py = mmp.tile([128, 2 * QC], F32, tag="mm",
                                          name=f"py{qc}_{ss}_{nn}")
                        else:
                            py = pyp.tile([128, QC], F32, tag="py",
                                          name=f"py{qc}_{ss}_{nn}")
                        for hp in range(2):
                            nc.tensor.matmul(
                                py[:, 0:QC],
                                ctx_sb[hp][:, ss * 128:(ss + 1) * 128],
                                two[:, hp, nn * QC:(nn + 1) * QC],
                                start=(hp == 0), stop=(hp == 1))
                        ysb = yp.tile([128, QC], BF16, tag="y",
                                      name=f"y{qc}_{ss}_{nn}")
                        if cfg.get("ycopy", "dve") == "act":
                            nc.scalar.copy(ysb[:], py[:, 0:QC])
                        else:
                            nc.vector.tensor_copy(ysb[:], py[:, 0:QC])
                        nc.sync.dma_start(
                            y[s0:s0 + 128, nn * QC:(nn + 1) * QC], ysb[:])

            # interleave projection blocks with attention q-chunks
            qc_order = cfg.get("qc_order", [0, 1, 2, 3])
            for blk in range(NQC):
                proj_wave(blk)
                if blk == 1:
                    out_proj(0, attention(0))
            for qc in qc_order[1:]:
                out_proj(qc, attention(qc), last=(qc == qc_order[-1]))

    nc.compile()
    return nc


def _get_nc():
    if "nc" not in _CACHE:
        _CACHE["nc"] = _build_nc()
    return _CACHE["nc"]


def make_mask():
    kl = np.arange(128)[:, None]
    ql = np.arange(128)[None, :]
    return (ql >= kl).astype(BF)


def shard_inputs(x, Wq, bq, Wk, bk, Wv, bv, Wo, bo):
    """Build the 8 per-core input maps (host-side sharding)."""
    x = np.asarray(x, dtype=np.float32)
    scale = np.float32(1.0 / np.sqrt(D))
    mask = make_mask()
    ones = np.ones((1, 64), BF)
    in_maps = []
    xTb = [np.ascontiguousarray(np.asarray(x[b]).T).astype(BF)
           for b in range(B)]
    for c in range(NCORES):
        b, g = divmod(c, 4)
        cs = slice(g * EC, (g + 1) * EC)
        in_maps.append({
            "xT": xTb[b],
            "wq": np.ascontiguousarray(
                (np.asarray(Wq[:, cs]) * scale)).astype(BF),
            "wk": np.ascontiguousarray(np.asarray(Wk[:, cs])).astype(BF),
            "wv": np.ascontiguousarray(np.asarray(Wv[:, cs])).astype(BF),
            "wo": np.ascontiguousarray(np.asarray(Wo[cs, :])).astype(BF),
            "bq": (np.asarray(bq[cs]) * scale).reshape(2, 128, 1)
                .astype(np.float32),
            "bk": np.asarray(bk[cs]).reshape(2, 128, 1).astype(np.float32),
            "bv": np.asarray(bv[cs]).reshape(1, EC).astype(np.float32),
            "msk": mask,
            "ones": ones,
        })
    return in_maps


def combine_outputs(results, bo):
    y = np.zeros((B, S, E), np.float32)
    for c in range(NCORES):
        b = c // 4
        y[b] += np.asarray(results[c]["y"], dtype=np.float32)
    y += np.asarray(bo, dtype=np.float32)[None, None, :]
    return y


def kernel(x, Wq, bq, Wk, bk, Wv, bv, Wo, bo):
    from concourse.bass_utils import run_bass_kernel_spmd

    nc = _get_nc()
    in_maps = shard_inputs(x, Wq, bq, Wk, bk, Wv, bv, Wo, bo)
    try:
        res = run_bass_kernel_spmd(nc, in_maps, core_ids=list(range(NCORES)))
    except Exception:
        # transient device errors (e.g. a wedged core) usually clear on retry
        res = run_bass_kernel_spmd(nc, in_maps, core_ids=list(range(NCORES)))
    return combine_outputs(res.results, bo)


# revision 44
# speedup vs baseline: 1.5590x; 1.3194x over previous
"""Causal self-attention (B=2, S=2048, E=1024, H=16) on 8 TRN2 NeuronCores.

Sharding: core c = 4*b + g handles batch b and head-group g (4 heads,
256 E-columns). Each core computes q/k/v projections for its head slice,
causal flash-style attention for its 4 heads, and a partial output
projection y_c = ctx_g @ Wo[rows_g].  Host sums the 4 partials per batch
and adds bo.

All matmul operands are bf16 (inputs converted on host); PSUM accumulation
stays f32.  xT streams in s-chunk-major order interleaved with split
weight loads so projection work starts as soon as the first pieces land.

Attention (q-major context accumulation):
  scoresT [kpos=128, q] per (q-chunk, head-pair, k-tile) -> ACT exp ->
  te bf16 -> gpsimd multiplies a 0/1 triangle into the diagonal block.
  ctx accumulates q-major: out[q=128, d=64] += te-subtile.T @ v-tile
  (te [128,128] stationary, v [128,64] moving -> only 64 rows/matmul);
  the softmax denominator accumulates via an extra 1-row matmul per
  stationary (moving operand = ones column).  All 4 q-subtiles x 2 heads
  of a (q-chunk, head-pair) pack into ONE PSUM bank as a single
  accumulation group; denominators share one bank across the kernel.
  Normalization: per-partition reciprocal [128,4,2] + one DVE multiply
  (denominator broadcast along d via stride-0 AP) -> ctx_n bf16.  PE
  transposes (identity matmul) flip ctx_n to d-major ctxT for the output
  projection.

Scheduling: the attention inner loop is ACT(exp)-throughput-bound, so the
next chunk's projection-wave units and the previous chunk's out_proj
units are emitted as "fillers" between attention (hp, k-tile) rounds --
PE fills its slack with them and every pool stall has independent work
behind it.  Causal trimming: for diagonal k-tile t' only q-columns >=
128*t' are computed (scores, exp, ctx all restricted).
"""

import os

import numpy as np
import ml_dtypes

os.environ.setdefault("NEURON_RT_RESET_CORES", "1")

B, S, E, H, D = 2, 2048, 1024, 16, 64
NCORES = 8
EC = 256          # E-columns per core (4 heads x 64)
QC = 512          # q-chunk width
NQC = S // QC     # 4
NKT = S // 128    # 16 k-tiles
NE = E // 128     # 8 contraction chunks

BF = ml_dtypes.bfloat16

_CACHE = {}


def _build_nc(cfg=None):
    cfg = cfg or {}
    MM_BUFS = cfg.get("mm", 2)
    PW_BUFS = cfg.get("pw", 2)
    EXP_BUFS = cfg.get("exp", 4)
    CTX_BUFS = cfg.get("ctx", 6)
    YP_BUFS = cfg.get("yp", 6)
    import concourse.mybir as mybir
    import concourse.tile as tile
    import concourse.bass as bass
    from concourse import bacc

    F32 = mybir.dt.float32
    BF16 = mybir.dt.bfloat16
    EXP = mybir.ActivationFunctionType.Exp

    nc = bacc.Bacc("TRN2", target_bir_lowering=False, debug=False)

    xT = nc.dram_tensor("xT", [E, S], BF16, kind="ExternalInput")
    wq = nc.dram_tensor("wq", [E, EC], BF16, kind="ExternalInput")
    wk = nc.dram_tensor("wk", [E, EC], BF16, kind="ExternalInput")
    wv = nc.dram_tensor("wv", [E, EC], BF16, kind="ExternalInput")
    wo = nc.dram_tensor("wo", [EC, E], BF16, kind="ExternalInput")
    bq = nc.dram_tensor("bq", [2, 128, 1], F32, kind="ExternalInput")
    bk = nc.dram_tensor("bk", [2, 128, 1], F32, kind="ExternalInput")
    bv = nc.dram_tensor("bv", [1, EC], F32, kind="ExternalInput")
    msk = nc.dram_tensor("msk", [128, 128], BF16, kind="ExternalInput")
    ones = nc.dram_tensor("ones", [128, 1], BF16, kind="ExternalInput")
    idn = nc.dram_tensor("idn", [128, 128], BF16, kind="ExternalInput")

    y = nc.dram_tensor("y", [S, E], BF16, kind="ExternalOutput")

    with tile.TileContext(nc) as tc:
        with (
            tc.tile_pool(name="weights", bufs=1) as wpool,
            tc.tile_pool(name="xtp", bufs=1) as xtp,
            tc.tile_pool(name="qkv", bufs=1) as qkv,
            tc.tile_pool(name="expp", bufs=EXP_BUFS) as expp,
            tc.tile_pool(name="ctxn", bufs=CTX_BUFS) as ctxp,
            tc.tile_pool(name="yp", bufs=YP_BUFS) as yp,
            tc.tile_pool(name="rows", bufs=3) as rows,
            tc.tile_pool(name="smalls", bufs=1) as smalls,
            tc.tile_pool(name="mm", bufs=MM_BUFS, space="PSUM") as mmp,
            tc.tile_pool(name="pw", bufs=PW_BUFS, space="PSUM") as pwp,
            tc.tile_pool(name="cx", bufs=1, space="PSUM") as cxp,
            tc.tile_pool(name="dnp", bufs=1, space="PSUM") as dnp,
        ):
            # ---- small constants (SWDGE/Pool queue; SP stays free) ----
            tbq = smalls.tile([128, 2], F32, tag="bq")
            tbk = smalls.tile([128, 2], F32, tag="bk")
            tbv = smalls.tile([128, EC], F32, tag="bv")
            tmsk = smalls.tile([128, 128], BF16, tag="msk")
            tones = smalls.tile([128, 1], BF16, tag="ones")
            tidn = smalls.tile([128, 128], BF16, tag="idn")

            for r in range(2):
                nc.gpsimd.dma_start(tbq[:, r:r + 1], bq[r])
                nc.gpsimd.dma_start(tbk[:, r:r + 1], bk[r])
            bvap = bv[0, :]
            bv_b = bass.AP(tensor=bvap.tensor, offset=bvap.offset,
                           ap=[[0, 128]] + list(bvap.ap))
            nc.gpsimd.dma_start(tbv[:], bv_b)
            nc.gpsimd.dma_start(tmsk[:], msk[:])
            nc.gpsimd.dma_start(tones[:], ones[:])
            nc.gpsimd.dma_start(tidn[:], idn[:])

            # ---- bulk inputs: s-chunk-major streaming ----
            twq = wpool.tile([128, NE, EC], BF16, tag="wq")
            twk = wpool.tile([128, NE, EC], BF16, tag="wk")
            twv = wpool.tile([128, NE, EC], BF16, tag="wv")
            two = wpool.tile([128, 2, E], BF16, tag="wo")

            def chunked_half(dram, width, h, nch=4):
                a = dram[:]
                return bass.AP(tensor=a.tensor,
                               offset=a.offset + h * nch * 128 * width,
                               ap=[[width, 128], [128 * width, nch],
                                   [1, width]])

            def chunked(dram, nch, width):
                a = dram[:]
                return bass.AP(tensor=a.tensor, offset=a.offset,
                               ap=[[width, 128], [128 * width, nch],
                                   [1, width]])

            txt = [xtp.tile([128, S], BF16, tag=f"xt{e}", name=f"xt{e}")
                   for e in range(NE)]

            def ldx(e, sc, q=None):
                (q or nc.sync).dma_start(
                    txt[e][:, sc * QC:(sc + 1) * QC],
                    xT[e * 128:(e + 1) * 128, sc * QC:(sc + 1) * QC])

            # wave-0 critical path, ordered by first consumer: the
            # e-interleaved wave-0 needs wq/wk/wv A-halves + x(0,0) before
            # its first e-step, then one x piece per step
            nc.sync.dma_start(twq[:, 0:4, :], chunked_half(wq, EC, 0))
            ldx(0, 0, nc.scalar)
            ldx(1, 0, nc.scalar)
            ldx(2, 0, nc.scalar)
            ldx(3, 0, nc.scalar)
            nc.sync.dma_start(twk[:, 0:4, :], chunked_half(wk, EC, 0))
            nc.sync.dma_start(twv[:, 0:4, :], chunked_half(wv, EC, 0))
            nc.sync.dma_start(twq[:, 4:8, :], chunked_half(wq, EC, 1))
            for e in range(4, NE):
                ldx(e, 0)
            nc.sync.dma_start(twk[:, 4:8, :], chunked_half(wk, EC, 1))
            nc.sync.dma_start(twv[:, 4:8, :], chunked_half(wv, EC, 1))
            for e in range(NE):
                ldx(e, 1)
            nc.sync.dma_start(two[:], chunked(wo, 2, E))
            for sc in range(2, NQC):
                for e in range(NE):
                    ldx(e, sc)

            # ---- persistent activation tiles ----
            tq = [qkv.tile([128, S], BF16, tag=f"q{r}", name=f"q{r}")
                  for r in range(2)]
            tk = [qkv.tile([128, S], BF16, tag=f"k{r}", name=f"k{r}")
                  for r in range(2)]
            tv = qkv.tile([128, NKT, 4, 64], BF16, tag="v")

            # broadcast tri-mask [128,128] over the two head-halves
            def mask_b(n):
                m = tmsk[:]
                return bass.AP(tensor=m.tensor, offset=m.offset,
                               ap=[list(m.ap[0]), [0, 2], [1, n]])

            def qk_unit(scn, w, wb, r, tdst):
                """One projection unit: dst[r-half, s-chunk scn] over 8
                contraction steps in one PSUM bank, drained by DVE."""
                sc = slice(scn * QC, (scn + 1) * QC)
                pt = pwp.tile([128, QC], F32, tag="pw",
                              name=f"u{scn}_{r}")
                for e in range(NE):
                    nc.tensor.matmul(
                        pt[:], w[:, e, r * 128:(r + 1) * 128],
                        txt[e][:, sc], start=(e == 0), stop=(e == NE - 1))
                nc.vector.tensor_scalar_add(
                    tdst[r][:, sc], pt[:], wb[:, r:r + 1])

            def v_unit(scn, i):
                v_ps = pwp.tile([128, 2, EC], F32, tag="pw",
                                name=f"pv{scn}_{i}")
                for e in range(NE):
                    for j in range(2):
                        st = 4 * scn + 2 * i + j
                        nc.tensor.matmul(
                            v_ps[:, j, :],
                            txt[e][:, st * 128:(st + 1) * 128],
                            twv[:, e, :],
                            start=(e == 0 and j == 0),
                            stop=(e == NE - 1 and j == 1))
                for j in range(2):
                    st = 4 * scn + 2 * i + j
                    nc.vector.tensor_add(
                        tv[:, st, :, :],
                        v_ps[:, j, :].rearrange("p (h d) -> p h d", h=4),
                        tbv[:].rearrange("p (h d) -> p h d", h=4))

            def qk_pieces(scn, w, wb, r, tdst):
                sc = slice(scn * QC, (scn + 1) * QC)
                cell = {}

                def p0():
                    cell["pt"] = pwp.tile([128, QC], F32, tag="pw",
                                          name=f"u{scn}_{r}")
                    for e in range(4):
                        nc.tensor.matmul(
                            cell["pt"][:], w[:, e, r * 128:(r + 1) * 128],
                            txt[e][:, sc], start=(e == 0), stop=False)

                def p1():
                    pt = cell["pt"]
                    for e in range(4, NE):
                        nc.tensor.matmul(
                            pt[:], w[:, e, r * 128:(r + 1) * 128],
                            txt[e][:, sc], start=False,
                            stop=(e == NE - 1))
                    nc.vector.tensor_scalar_add(
                        tdst[r][:, sc], pt[:], wb[:, r:r + 1])
                return [p0, p1]

            def v_pieces(scn, i):
                cell = {}

                def mk(elo, ehi):
                    def p():
                        if elo == 0:
                            cell["pt"] = pwp.tile(
                                [128, 2, EC], F32, tag="pw",
                                name=f"pv{scn}_{i}")
                        v_ps = cell["pt"]
                        for e in range(elo, ehi):
                            for j in range(2):
                                st = 4 * scn + 2 * i + j
                                nc.tensor.matmul(
                                    v_ps[:, j, :],
                                    txt[e][:, st * 128:(st + 1) * 128],
                                    twv[:, e, :],
                                    start=(e == 0 and j == 0),
                                    stop=(e == NE - 1 and j == 1))
                        if ehi == NE:
                            for j in range(2):
                                st = 4 * scn + 2 * i + j
                                nc.vector.tensor_add(
                                    tv[:, st, :, :],
                                    v_ps[:, j, :].rearrange(
                                        "p (h d) -> p h d", h=4),
                                    tbv[:].rearrange(
                                        "p (h d) -> p h d", h=4))
                    return p
                return [mk(0, 4), mk(4, NE)]

            def unit(scn, which):
                fine = cfg.get("fine", True)
                if which == "q0":
                    ps_ = qk_pieces(scn, twq, tbq, 0, tq)
                elif which == "k0":
                    ps_ = qk_pieces(scn, twk, tbk, 0, tk)
                elif which == "q1":
                    ps_ = qk_pieces(scn, twq, tbq, 1, tq)
                elif which == "k1":
                    ps_ = qk_pieces(scn, twk, tbk, 1, tk)
                elif which == "v0":
                    ps_ = v_pieces(scn, 0)
                else:
                    ps_ = v_pieces(scn, 1)
                if fine:
                    return ps_
                return [lambda: [p() for p in ps_]]

            YENG = cfg.get("yeng", ["dve", "pool", "dve", "pool"])

            def op_pieces(qc, ss, ctxT, pool="pw"):
                """Output projection for s-subtile ss of chunk qc as two
                filler pieces, each a [128, QC] PSUM half in the pw pool
                (the mm pool stays exclusive to the scores stream)."""
                s0 = qc * QC + ss * 128

                cells = [{}, {}]

                def mk(nn, hp):
                    def p():
                        if hp == 0:
                            # tail chunk: attention is over, so the scores
                            # pool is free -- alternate pw/mm slots to get
                            # more projection pieces in flight
                            tail = (qc == NQC - 1
                                    and cfg.get("tailmm", True))
                            pool = mmp if (tail and nn == 0) else pwp
                            tg = "mm" if (tail and nn == 0) else "pw"
                            cells[nn]["py"] = pool.tile(
                                [128, QC], F32, tag=tg,
                                name=f"py{qc}_{ss}_{nn}")
                        py = cells[nn]["py"]
                        nc.tensor.matmul(
                            py[:],
                            ctxT[hp][:, ss, :],
                            two[:, hp, nn * QC:(nn + 1) * QC],
                            start=(hp == 0), stop=(hp == 1))
                        if hp == 0:
                            return
                        ysb = yp.tile([128, QC], BF16, tag="y",
                                      name=f"y{qc}_{ss}_{nn}")
                        # GPSIMD cannot read PSUM on hardware: staging
                        # copies go to DVE, or ACT at the tail (its exp
                        # stream is done by then)
                        if qc == NQC - 1 and (2 * ss + nn) % 2 == 1:
                            nc.scalar.copy(ysb[:], py[:])
                        else:
                            nc.vector.tensor_copy(ysb[:], py[:])
                        q = nc.scalar if (qc == NQC - 1 and nn == 1) \
                            else nc.sync
                        q.dma_start(
                            y[s0:s0 + 128, nn * QC:(nn + 1) * QC], ysb[:])
                    return p
                if cfg.get("ophp", False):
                    return [mk(0, 0), mk(0, 1), mk(1, 0), mk(1, 1)]
                return [lambda a=mk(0, 0), b=mk(0, 1): (a(), b()),
                        lambda a=mk(1, 0), b=mk(1, 1): (a(), b())]

            def attention(qc, fillers):
                """fillers: emit-closures (prev chunk's out_proj + next
                chunk's projection units) popped between (hp, kt) rounds
                so PE slack under the ACT-bound exp stream is used."""
                n_kt = 4 * (qc + 1)
                rounds_total = 2 * (n_kt + 1)
                n_fill0 = len(fillers)
                state = {"round": 0, "popped": 0}

                def pop_fillers():
                    state["round"] += 1
                    want = state["round"] * n_fill0 // rounds_total
                    while fillers and (
                            state["popped"] < want
                            or fillers[0][1] <= state["round"]):
                        fillers.pop(0)[0]()
                        state["popped"] += 1

                dn = dnp.tile([128, 2, 4, 2], F32, tag="dn",
                              name=f"dn{qc}")
                ctxT = [None, None]
                for hp in range(2):
                    pctx = cxp.tile([128, 4, 2, 64], F32, tag="cx",
                                    name=f"cx{qc}_{hp}")

                    def ctx_round(kt, te):
                        dg = kt - 4 * qc
                        sub0 = dg if dg > 0 else 0
                        for sub in range(sub0, 4):
                            for h2 in range(2):
                                h = 2 * hp + h2
                                st_ap = te[:, h2 * QC + sub * 128:
                                           h2 * QC + (sub + 1) * 128]
                                first = (kt == 0 and sub == 0 and h2 == 0)
                                last = (kt == n_kt - 1 and sub == 3
                                        and h2 == 1)
                                nc.tensor.matmul(
                                    pctx[:, sub, h2, :], st_ap,
                                    tv[:, kt, h, :],
                                    start=first, stop=last)
                                nc.tensor.matmul(
                                    dn[:, hp, sub, h2:h2 + 1], st_ap,
                                    tones[:],
                                    start=first, stop=last)

                    prev = None
                    for kt in range(n_kt + 1):
                        if kt < n_kt:
                            dg = kt - 4 * qc  # >=0: diagonal tile index
                            coff = 128 * dg if dg > 0 else 0
                            ps = mmp.tile([128, 2 * QC], F32, tag="mm",
                                          name=f"ps{qc}_{hp}_{kt}")
                            te = expp.tile([128, 2 * QC], BF16, tag="exp",
                                           name=f"te{qc}_{hp}_{kt}")
                            for h2 in range(2):
                                bp = h2 * 64
                                nc.tensor.matmul(
                                    ps[:, h2 * QC + coff:(h2 + 1) * QC],
                                    tk[hp][bp:bp + 64,
                                           kt * 128:(kt + 1) * 128],
                                    tq[hp][bp:bp + 64,
                                           qc * QC + coff:(qc + 1) * QC],
                                    start=True, stop=True)
                            if coff:
                                ps3 = ps[:].rearrange("p (t n) -> p t n",
                                                      t=2)
                                te3 = te[:].rearrange("p (t n) -> p t n",
                                                      t=2)
                                nc.scalar.activation(
                                    te3[:, :, coff:QC], ps3[:, :, coff:QC],
                                    EXP)
                            else:
                                nc.scalar.activation(te[:], ps[:], EXP)
                            if dg >= 0:
                                te3 = te[:].rearrange("p (t n) -> p t n",
                                                      t=2)
                                nc.gpsimd.tensor_mul(
                                    te3[:, :, coff:coff + 128],
                                    te3[:, :, coff:coff + 128],
                                    mask_b(128))
                        else:
                            te = None
                        if prev is not None:
                            ctx_round(*prev)
                        prev = (kt, te) if te is not None else None
                        pop_fillers()
                    # ---- normalization + transpose to d-major ----
                    rec = rows.tile([128, 4, 2], F32, tag="rec")
                    nc.vector.reciprocal(rec[:], dn[:, hp])
                    ctx_n = ctxp.tile([128, 4, 2, 64], BF16, tag="ctxn",
                                      name=f"cn{qc}_{hp}")
                    r_ap = rec[:]
                    rec_b = bass.AP(tensor=r_ap.tensor, offset=r_ap.offset,
                                    ap=[list(r_ap.ap[0]), [2, 4], [1, 2],
                                        [0, 64]])
                    nc.vector.tensor_mul(ctx_n[:], pctx[:], rec_b)
                    ctxT_sb = ctxp.tile([128, 4, 128], BF16, tag="ctxn",
                                        name=f"ct{qc}_{hp}")
                    if qc < NQC - 1 and not cfg.get("petp", False):
                        # SBUF->SBUF crossbar transpose: keeps PE and the
                        # pw PSUM pool out of the normalization chain
                        for sub in range(4):
                            nc.sync.dma_start_transpose(
                                ctxT_sb[:, sub, :], ctx_n[:, sub])
                    else:
                        # tail chunk: PE transpose (pw pool is idle by
                        # then, and latency beats the DMA path)
                        tp = pwp.tile([128, 4, 128], BF16, tag="pw",
                                      name=f"tp{qc}_{hp}")
                        for sub in range(4):
                            nc.tensor.transpose(
                                tp[:, sub, :], ctx_n[:, sub], tidn[:])
                        nc.vector.tensor_copy(ctxT_sb[:], tp[:])
                    ctxT[hp] = ctxT_sb
                while fillers:
                    fillers.pop(0)[0]()
                return ctxT

            # chunk-0 hp0 prerequisites emitted compactly; everything else
            # (hp1 halves, later chunks' units, out_proj) flows as fillers.
            # wave-0 runs e-interleaved across all four units (q0/k0 from
            # the still-idle mm pool, v pairs from pw): consumes each
            # arriving x-piece with ~850ns of PE work, so the startup is
            # DMA-paced with minimal PE idling
            w0q = mmp.tile([128, QC], F32, tag="mm", name="w0q")
            w0k = mmp.tile([128, QC], F32, tag="mm", name="w0k")
            w0v = [pwp.tile([128, 2, EC], F32, tag="pw", name=f"w0v{i}")
                   for i in range(2)]
            for e in range(NE):
                nc.tensor.matmul(w0q[:], twq[:, e, 0:128], txt[e][:, 0:QC],
                                 start=(e == 0), stop=(e == NE - 1))
                nc.tensor.matmul(w0k[:], twk[:, e, 0:128], txt[e][:, 0:QC],
                                 start=(e == 0), stop=(e == NE - 1))
                for i in range(2):
                    for j in range(2):
                        st = 2 * i + j
                        nc.tensor.matmul(
                            w0v[i][:, j, :],
                            txt[e][:, st * 128:(st + 1) * 128],
                            twv[:, e, :],
                            start=(e == 0 and j == 0),
                            stop=(e == NE - 1 and j == 1))
            nc.vector.tensor_scalar_add(tq[0][:, 0:QC], w0q[:], tbq[:, 0:1])
            nc.vector.tensor_scalar_add(tk[0][:, 0:QC], w0k[:], tbk[:, 0:1])
            for i in range(2):
                for j in range(2):
                    st = 2 * i + j
                    nc.vector.tensor_add(
                        tv[:, st, :, :],
                        w0v[i][:, j, :].rearrange("p (h d) -> p h d", h=4),
                        tbv[:].rearrange("p (h d) -> p h d", h=4))
            pend = []
            ctxT = None
            for qc in range(NQC):
                # fillers: (closure, deadline-round) -- deadlines force
                # emission before the attention rounds that consume them
                n_kt = 4 * (qc + 1)
                fillers = []
                if qc > 0:
                    # this chunk's k/v are only read from its diagonal
                    # k-tiles (round 4*qc) on; its q1/k1 only from hp1
                    for w, dl in (("k0", 4 * qc), ("v0", 4 * qc + 1),
                                  ("v1", 4 * qc + 1), ("q1", n_kt + 1),
                                  ("k1", n_kt + 1)):
                        fillers += [(p, dl) for p in unit(qc, w)]
                else:
                    fillers += [(p, n_kt + 1)
                                for p in unit(0, "q1") + unit(0, "k1")]
                if qc == NQC - 1:
                    # last chunk has no projection fillers: pin the
                    # out_proj pieces to its late (otherwise ACT-starved)
                    # rounds instead of letting them drain early
                    nr = 2 * (n_kt + 1)
                    step = max(1, (nr - n_kt - 2) // max(1, len(pend)))
                    fillers += [(p, n_kt + 2 + i * step)
                                for i, p in enumerate(pend)]
                else:
                    fillers += [(p, 10 ** 9) for p in pend]
                if qc + 1 < NQC:
                    fillers += [(p, 10 ** 9) for p in unit(qc + 1, "q0")]
                ctxT = attention(qc, fillers)
                pend = []
                for ss in range(4):
                    pend += op_pieces(qc, ss, ctxT)
            for fn in pend:
                fn()

    nc.compile()
    return nc


def _get_nc():
    if "nc" not in _CACHE:
        _CACHE["nc"] = _build_nc()
    return _CACHE["nc"]


def make_mask():
    kl = np.arange(128)[:, None]
    ql = np.arange(128)[None, :]
    return (ql >= kl).astype(BF)


def shard_inputs(x, Wq, bq, Wk, bk, Wv, bv, Wo, bo):
    """Build the 8 per-core input maps (host-side sharding)."""
    x = np.asarray(x, dtype=np.float32)
    scale = np.float32(1.0 / np.sqrt(D))
    mask = make_mask()
    ones = np.ones((128, 1), BF)
    idn = np.eye(128, dtype=np.float32).astype(BF)
    in_maps = []
    xTb = [np.ascontiguousarray(np.asarray(x[b]).T).astype(BF)
           for b in range(B)]
    for c in range(NCORES):
        b, g = divmod(c, 4)
        cs = slice(g * EC, (g + 1) * EC)
        in_maps.append({
            "xT": xTb[b],
            "wq": np.ascontiguousarray(
                (np.asarray(Wq[:, cs]) * scale)).astype(BF),
            "wk": np.ascontiguousarray(np.asarray(Wk[:, cs])).astype(BF),
            "wv": np.ascontiguousarray(np.asarray(Wv[:, cs])).astype(BF),
            "wo": np.ascontiguousarray(np.asarray(Wo[cs, :])).astype(BF),
            "bq": (np.asarray(bq[cs]) * scale).reshape(2, 128, 1)
                .astype(np.float32),
            "bk": np.asarray(bk[cs]).reshape(2, 128, 1).astype(np.float32),
            "bv": np.asarray(bv[cs]).reshape(1, EC).astype(np.float32),
            "msk": mask,
            "ones": ones,
            "idn": idn,
        })
    return in_maps


def combine_outputs(results, bo):
    y = np.zeros((B, S, E), np.float32)
    for c in range(NCORES):
        b = c // 4
        y[b] += np.asarray(results[c]["y"], dtype=np.float32)
    y += np.asarray(bo, dtype=np.float32)[None, None, :]
    return y


def kernel(x, Wq, bq, Wk, bk, Wv, bv, Wo, bo):
    from concourse.bass_utils import run_bass_kernel_spmd

    nc = _get_nc()
    in_maps = shard_inputs(x, Wq, bq, Wk, bk, Wv, bv, Wo, bo)
    try:
        res = run_bass_kernel_spmd(nc, in_maps, core_ids=list(range(NCORES)))
    except Exception:
        # transient device errors (e.g. a wedged core) usually clear on retry
        res = run_bass_kernel_spmd(nc, in_maps, core_ids=list(range(NCORES)))
    return combine_outputs(res.results, bo)


# revision 52
# speedup vs baseline: 1.5730x; 1.0090x over previous
"""Causal self-attention (B=2, S=2048, E=1024, H=16) on 8 TRN2 NeuronCores.

Sharding: core c = 4*b + g handles batch b and head-group g (4 heads,
256 E-columns). Each core computes q/k/v projections for its head slice,
causal flash-style attention for its 4 heads, and a partial output
projection y_c = ctx_g @ Wo[rows_g].  Host sums the 4 partials per batch
and adds bo.

All matmul operands are bf16 (inputs converted on host); PSUM accumulation
stays f32.  xT streams in s-chunk-major order interleaved with split
weight loads so projection work starts as soon as the first pieces land.

Attention (q-major context accumulation):
  scoresT [kpos=128, q] per (q-chunk, head-pair, k-tile) -> ACT exp ->
  te bf16 -> gpsimd multiplies a 0/1 triangle into the diagonal block.
  ctx accumulates q-major: out[q=128, d=64] += te-subtile.T @ v-tile
  (te [128,128] stationary, v [128,64] moving -> only 64 rows/matmul);
  the softmax denominator accumulates via an extra 1-row matmul per
  stationary (moving operand = ones column).  All 4 q-subtiles x 2 heads
  of a (q-chunk, head-pair) pack into ONE PSUM bank as a single
  accumulation group; denominators share one bank across the kernel.
  Normalization: per-partition reciprocal [128,4,2] + one DVE multiply
  (denominator broadcast along d via stride-0 AP) -> ctx_n bf16.  PE
  transposes (identity matmul) flip ctx_n to d-major ctxT for the output
  projection.

Scheduling: the attention inner loop is ACT(exp)-throughput-bound, so the
next chunk's projection-wave units and the previous chunk's out_proj
units are emitted as "fillers" between attention (hp, k-tile) rounds --
PE fills its slack with them and every pool stall has independent work
behind it.  Causal trimming: for diagonal k-tile t' only q-columns >=
128*t' are computed (scores, exp, ctx all restricted).
"""

import os

import numpy as np
import ml_dtypes

os.environ.setdefault("NEURON_RT_RESET_CORES", "1")

B, S, E, H, D = 2, 2048, 1024, 16, 64
NCORES = 8
EC = 256          # E-columns per core (4 heads x 64)
QC = 512          # q-chunk width
NQC = S // QC     # 4
NKT = S // 128    # 16 k-tiles
NE = E // 128     # 8 contraction chunks

BF = ml_dtypes.bfloat16

_CACHE = {}


def _build_nc(cfg=None):
    cfg = cfg or {}
    MM_BUFS = cfg.get("mm", 2)
    PW_BUFS = cfg.get("pw", 2)
    EXP_BUFS = cfg.get("exp", 4)
    CTX_BUFS = cfg.get("ctx", 6)
    YP_BUFS = cfg.get("yp", 6)
    import concourse.mybir as mybir
    import concourse.tile as tile
    import concourse.bass as bass
    from concourse import bacc

    F32 = mybir.dt.float32
    BF16 = mybir.dt.bfloat16
    EXP = mybir.ActivationFunctionType.Exp

    nc = bacc.Bacc("TRN2", target_bir_lowering=False, debug=False)

    xT = nc.dram_tensor("xT", [E, S], BF16, kind="ExternalInput")
    wq = nc.dram_tensor("wq", [E, EC], BF16, kind="ExternalInput")
    wk = nc.dram_tensor("wk", [E, EC], BF16, kind="ExternalInput")
    wv = nc.dram_tensor("wv", [E, EC], BF16, kind="ExternalInput")
    wo = nc.dram_tensor("wo", [EC, E], BF16, kind="ExternalInput")
    bq = nc.dram_tensor("bq", [2, 128, 1], F32, kind="ExternalInput")
    bk = nc.dram_tensor("bk", [2, 128, 1], F32, kind="ExternalInput")
    bv = nc.dram_tensor("bv", [1, EC], F32, kind="ExternalInput")
    msk = nc.dram_tensor("msk", [128, 128], BF16, kind="ExternalInput")
    ones = nc.dram_tensor("ones", [128, 1], BF16, kind="ExternalInput")
    idn = nc.dram_tensor("idn", [128, 128], BF16, kind="ExternalInput")

    y = nc.dram_tensor("y", [S, E], BF16, kind="ExternalOutput")

    with tile.TileContext(nc) as tc:
        with (
            tc.tile_pool(name="weights", bufs=1) as wpool,
            tc.tile_pool(name="xtp", bufs=1) as xtp,
            tc.tile_pool(name="qkv", bufs=1) as qkv,
            tc.tile_pool(name="expp", bufs=EXP_BUFS) as expp,
            tc.tile_pool(name="ctxn", bufs=CTX_BUFS) as ctxp,
            tc.tile_pool(name="yp", bufs=YP_BUFS) as yp,
            tc.tile_pool(name="rows", bufs=3) as rows,
            tc.tile_pool(name="smalls", bufs=1) as smalls,
            tc.tile_pool(name="mm", bufs=MM_BUFS, space="PSUM") as mmp,
            tc.tile_pool(name="pw", bufs=PW_BUFS, space="PSUM") as pwp,
            tc.tile_pool(name="cx", bufs=1, space="PSUM") as cxp,
            tc.tile_pool(name="dnp", bufs=1, space="PSUM") as dnp,
        ):
            # ---- small constants (SWDGE/Pool queue; SP stays free) ----
            tbq = smalls.tile([128, 2], F32, tag="bq")
            tbk = smalls.tile([128, 2], F32, tag="bk")
            tbv = smalls.tile([128, EC], F32, tag="bv")
            tmsk = smalls.tile([128, 128], BF16, tag="msk")
            tones = smalls.tile([128, 1], BF16, tag="ones")
            tidn = smalls.tile([128, 128], BF16, tag="idn")

            for r in range(2):
                nc.gpsimd.dma_start(tbq[:, r:r + 1], bq[r])
                nc.gpsimd.dma_start(tbk[:, r:r + 1], bk[r])
            bvap = bv[0, :]
            bv_b = bass.AP(tensor=bvap.tensor, offset=bvap.offset,
                           ap=[[0, 128]] + list(bvap.ap))
            nc.gpsimd.dma_start(tbv[:], bv_b)
            nc.gpsimd.dma_start(tmsk[:], msk[:])
            nc.gpsimd.dma_start(tones[:], ones[:])
            nc.gpsimd.dma_start(tidn[:], idn[:])

            # ---- bulk inputs: s-chunk-major streaming ----
            twq = wpool.tile([128, NE, EC], BF16, tag="wq")
            twk = wpool.tile([128, NE, EC], BF16, tag="wk")
            twv = wpool.tile([128, NE, EC], BF16, tag="wv")
            two = wpool.tile([128, 2, E], BF16, tag="wo")

            def chunked_half(dram, width, h, nch=4):
                a = dram[:]
                return bass.AP(tensor=a.tensor,
                               offset=a.offset + h * nch * 128 * width,
                               ap=[[width, 128], [128 * width, nch],
                                   [1, width]])

            def chunked(dram, nch, width):
                a = dram[:]
                return bass.AP(tensor=a.tensor, offset=a.offset,
                               ap=[[width, 128], [128 * width, nch],
                                   [1, width]])

            txt = [xtp.tile([128, S], BF16, tag=f"xt{e}", name=f"xt{e}")
                   for e in range(NE)]

            def ldx(e, sc, q=None):
                (q or nc.sync).dma_start(
                    txt[e][:, sc * QC:(sc + 1) * QC],
                    xT[e * 128:(e + 1) * 128, sc * QC:(sc + 1) * QC])

            # wave-0 critical path, ordered by first consumer: the
            # e-interleaved wave-0 needs wq/wk/wv A-halves + x(0,0) before
            # its first e-step, then one x piece per step
            nc.sync.dma_start(twq[:, 0:2, :], chunked_half(wq, EC, 0, 2))
            ldx(0, 0, nc.scalar)
            ldx(1, 0, nc.scalar)
            nc.sync.dma_start(
                twq[:, 2:4, :],
                bass.AP(tensor=wq[:].tensor,
                        offset=wq[:].offset + 2 * 128 * EC,
                        ap=[[EC, 128], [128 * EC, 2], [1, EC]]))
            ldx(2, 0, nc.scalar)
            ldx(3, 0, nc.scalar)
            nc.sync.dma_start(twk[:, 0:4, :], chunked_half(wk, EC, 0))
            nc.sync.dma_start(twv[:, 0:4, :], chunked_half(wv, EC, 0))
            nc.sync.dma_start(twq[:, 4:8, :], chunked_half(wq, EC, 1))
            for e in range(4, NE):
                ldx(e, 0)
            nc.sync.dma_start(twk[:, 4:8, :], chunked_half(wk, EC, 1))
            nc.sync.dma_start(twv[:, 4:8, :], chunked_half(wv, EC, 1))
            for e in range(NE):
                ldx(e, 1)
            nc.sync.dma_start(two[:], chunked(wo, 2, E))
            for sc in range(2, NQC):
                for e in range(NE):
                    ldx(e, sc)

            # ---- persistent activation tiles ----
            tq = [qkv.tile([128, S], BF16, tag=f"q{r}", name=f"q{r}")
                  for r in range(2)]
            tk = [qkv.tile([128, S], BF16, tag=f"k{r}", name=f"k{r}")
                  for r in range(2)]
            tv = qkv.tile([128, NKT, 4, 64], BF16, tag="v")

            # broadcast tri-mask [128,128] over the two head-halves
            def mask_b(n):
                m = tmsk[:]
                return bass.AP(tensor=m.tensor, offset=m.offset,
                               ap=[list(m.ap[0]), [0, 2], [1, n]])

            def qk_unit(scn, w, wb, r, tdst):
                """One projection unit: dst[r-half, s-chunk scn] over 8
                contraction steps in one PSUM bank, drained by DVE."""
                sc = slice(scn * QC, (scn + 1) * QC)
                pt = pwp.tile([128, QC], F32, tag="pw",
                              name=f"u{scn}_{r}")
                for e in range(NE):
                    nc.tensor.matmul(
                        pt[:], w[:, e, r * 128:(r + 1) * 128],
                        txt[e][:, sc], start=(e == 0), stop=(e == NE - 1))
                nc.vector.tensor_scalar_add(
                    tdst[r][:, sc], pt[:], wb[:, r:r + 1])

            def v_unit(scn, i):
                v_ps = pwp.tile([128, 2, EC], F32, tag="pw",
                                name=f"pv{scn}_{i}")
                for e in range(NE):
                    for j in range(2):
                        st = 4 * scn + 2 * i + j
                        nc.tensor.matmul(
                            v_ps[:, j, :],
                            txt[e][:, st * 128:(st + 1) * 128],
                            twv[:, e, :],
                            start=(e == 0 and j == 0),
                            stop=(e == NE - 1 and j == 1))
                for j in range(2):
                    st = 4 * scn + 2 * i + j
                    nc.vector.tensor_add(
                        tv[:, st, :, :],
                        v_ps[:, j, :].rearrange("p (h d) -> p h d", h=4),
                        tbv[:].rearrange("p (h d) -> p h d", h=4))

            def qk_pieces(scn, w, wb, r, tdst):
                sc = slice(scn * QC, (scn + 1) * QC)
                cell = {}

                def p0():
                    cell["pt"] = pwp.tile([128, QC], F32, tag="pw",
                                          name=f"u{scn}_{r}")
                    for e in range(4):
                        nc.tensor.matmul(
                            cell["pt"][:], w[:, e, r * 128:(r + 1) * 128],
                            txt[e][:, sc], start=(e == 0), stop=False)

                def p1():
                    pt = cell["pt"]
                    for e in range(4, NE):
                        nc.tensor.matmul(
                            pt[:], w[:, e, r * 128:(r + 1) * 128],
                            txt[e][:, sc], start=False,
                            stop=(e == NE - 1))
                    nc.vector.tensor_scalar_add(
                        tdst[r][:, sc], pt[:], wb[:, r:r + 1])
                return [p0, p1]

            def v_pieces(scn, i):
                cell = {}

                def mk(elo, ehi):
                    def p():
                        if elo == 0:
                            cell["pt"] = pwp.tile(
                                [128, 2, EC], F32, tag="pw",
                                name=f"pv{scn}_{i}")
                        v_ps = cell["pt"]
                        for e in range(elo, ehi):
                            for j in range(2):
                                st = 4 * scn + 2 * i + j
                                nc.tensor.matmul(
                                    v_ps[:, j, :],
                                    txt[e][:, st * 128:(st + 1) * 128],
                                    twv[:, e, :],
                                    start=(e == 0 and j == 0),
                                    stop=(e == NE - 1 and j == 1))
                        if ehi == NE:
                            for j in range(2):
                                st = 4 * scn + 2 * i + j
                                nc.vector.tensor_add(
                                    tv[:, st, :, :],
                                    v_ps[:, j, :].rearrange(
                                        "p (h d) -> p h d", h=4),
                                    tbv[:].rearrange(
                                        "p (h d) -> p h d", h=4))
                    return p
                return [mk(0, 4), mk(4, NE)]

            def unit(scn, which):
                fine = cfg.get("fine", True)
                if which == "q0":
                    ps_ = qk_pieces(scn, twq, tbq, 0, tq)
                elif which == "k0":
                    ps_ = qk_pieces(scn, twk, tbk, 0, tk)
                elif which == "q1":
                    ps_ = qk_pieces(scn, twq, tbq, 1, tq)
                elif which == "k1":
                    ps_ = qk_pieces(scn, twk, tbk, 1, tk)
                elif which == "v0":
                    ps_ = v_pieces(scn, 0)
                else:
                    ps_ = v_pieces(scn, 1)
                if fine:
                    return ps_
                return [lambda: [p() for p in ps_]]

            YENG = cfg.get("yeng", ["dve", "pool", "dve", "pool"])

            def op_pieces(qc, ss, ctxT, last=False):
                """Output projection for s-subtile ss of chunk qc as two
                filler pieces, each a [128, QC] PSUM half in the pw pool
                (the mm pool stays exclusive to the scores stream)."""
                s0 = qc * QC + ss * 128

                cells = [{}, {}]

                def mk(nn, hp):
                    def p():
                        if hp == 0:
                            # tail chunk: attention is over, so the scores
                            # pool is free -- alternate pw/mm slots to get
                            # more projection pieces in flight
                            tail = last and cfg.get("tailmm", True)
                            pool = mmp if (tail and nn == 0) else pwp
                            tg = "mm" if (tail and nn == 0) else "pw"
                            cells[nn]["py"] = pool.tile(
                                [128, QC], F32, tag=tg,
                                name=f"py{qc}_{ss}_{nn}")
                        py = cells[nn]["py"]
                        nc.tensor.matmul(
                            py[:],
                            ctxT[hp][:, ss, :],
                            two[:, hp, nn * QC:(nn + 1) * QC],
                            start=(hp == 0), stop=(hp == 1))
                        if hp == 0:
                            return
                        ysb = yp.tile([128, QC], BF16, tag="y",
                                      name=f"y{qc}_{ss}_{nn}")
                        # GPSIMD cannot read PSUM on hardware: staging
                        # copies go to DVE, or ACT at the tail (its exp
                        # stream is done by then)
                        if last and (2 * ss + nn) % 2 == 1:
                            nc.scalar.copy(ysb[:], py[:])
                        else:
                            nc.vector.tensor_copy(ysb[:], py[:])
                        q = nc.scalar if (last and nn == 1) \
                            else nc.sync
                        q.dma_start(
                            y[s0:s0 + 128, nn * QC:(nn + 1) * QC], ysb[:])
                    return p
                return [lambda a=mk(0, 0), b=mk(0, 1): (a(), b()),
                        lambda a=mk(1, 0), b=mk(1, 1): (a(), b())]

            def attention(qc, fillers, last=False):
                """fillers: emit-closures (prev chunk's out_proj + next
                chunk's projection units) popped between (hp, kt) rounds
                so PE slack under the ACT-bound exp stream is used."""
                n_kt = 4 * (qc + 1)
                rounds_total = 2 * (n_kt + 1)
                n_fill0 = len(fillers)
                state = {"round": 0, "popped": 0}

                def pop_fillers():
                    state["round"] += 1
                    want = state["round"] * n_fill0 // rounds_total
                    while fillers and (
                            state["popped"] < want
                            or fillers[0][1] <= state["round"]):
                        fillers.pop(0)[0]()
                        state["popped"] += 1

                dn = dnp.tile([128, 2, 4, 2], F32, tag="dn",
                              name=f"dn{qc}")
                ctxT = [None, None]
                for hp in range(2):
                    pctx = cxp.tile([128, 4, 2, 64], F32, tag="cx",
                                    name=f"cx{qc}_{hp}")

                    def ctx_round(kt, te):
                        dg = kt - 4 * qc
                        sub0 = dg if dg > 0 else 0
                        for sub in range(sub0, 4):
                            for h2 in range(2):
                                h = 2 * hp + h2
                                st_ap = te[:, h2 * QC + sub * 128:
                                           h2 * QC + (sub + 1) * 128]
                                first = (kt == 0 and sub == 0 and h2 == 0)
                                last = (kt == n_kt - 1 and sub == 3
                                        and h2 == 1)
                                nc.tensor.matmul(
                                    pctx[:, sub, h2, :], st_ap,
                                    tv[:, kt, h, :],
                                    start=first, stop=last)
                                nc.tensor.matmul(
                                    dn[:, hp, sub, h2:h2 + 1], st_ap,
                                    tones[:],
                                    start=first, stop=last)

                    prev = None
                    for kt in range(n_kt + 1):
                        if kt < n_kt:
                            dg = kt - 4 * qc  # >=0: diagonal tile index
                            coff = 128 * dg if dg > 0 else 0
                            ps = mmp.tile([128, 2 * QC], F32, tag="mm",
                                          name=f"ps{qc}_{hp}_{kt}")
                            te = expp.tile([128, 2 * QC], BF16, tag="exp",
                                           name=f"te{qc}_{hp}_{kt}")
                            for h2 in range(2):
                                bp = h2 * 64
                                nc.tensor.matmul(
                                    ps[:, h2 * QC + coff:(h2 + 1) * QC],
                                    tk[hp][bp:bp + 64,
                                           kt * 128:(kt + 1) * 128],
                                    tq[hp][bp:bp + 64,
                                           qc * QC + coff:(qc + 1) * QC],
                                    start=True, stop=True)
                            if coff:
                                ps3 = ps[:].rearrange("p (t n) -> p t n",
                                                      t=2)
                                te3 = te[:].rearrange("p (t n) -> p t n",
                                                      t=2)
                                nc.scalar.activation(
                                    te3[:, :, coff:QC], ps3[:, :, coff:QC],
                                    EXP)
                            else:
                                nc.scalar.activation(te[:], ps[:], EXP)
                            if dg >= 0:
                                te3 = te[:].rearrange("p (t n) -> p t n",
                                                      t=2)
                                nc.gpsimd.tensor_mul(
                                    te3[:, :, coff:coff + 128],
                                    te3[:, :, coff:coff + 128],
                                    mask_b(128))
                        else:
                            te = None
                        if prev is not None:
                            ctx_round(*prev)
                        prev = (kt, te) if te is not None else None
                        pop_fillers()
                    # ---- normalization + transpose to d-major ----
                    rec = rows.tile([128, 4, 2], F32, tag="rec")
                    nc.vector.reciprocal(rec[:], dn[:, hp])
                    ctx_n = ctxp.tile([128, 4, 2, 64], BF16, tag="ctxn",
                                      name=f"cn{qc}_{hp}")
                    r_ap = rec[:]
                    rec_b = bass.AP(tensor=r_ap.tensor, offset=r_ap.offset,
                                    ap=[list(r_ap.ap[0]), [2, 4], [1, 2],
                                        [0, 64]])
                    nc.vector.tensor_mul(ctx_n[:], pctx[:], rec_b)
                    ctxT_sb = ctxp.tile([128, 4, 128], BF16, tag="ctxn",
                                        name=f"ct{qc}_{hp}")
                    if not last and not cfg.get("petp", False):
                        # SBUF->SBUF crossbar transpose: keeps PE and the
                        # pw PSUM pool out of the normalization chain
                        for sub in range(4):
                            nc.sync.dma_start_transpose(
                                ctxT_sb[:, sub, :], ctx_n[:, sub])
                    else:
                        # tail chunk: PE transpose (pw pool is idle by
                        # then, and latency beats the DMA path)
                        tp = pwp.tile([128, 4, 128], BF16, tag="pw",
                                      name=f"tp{qc}_{hp}")
                        for sub in range(4):
                            nc.tensor.transpose(
                                tp[:, sub, :], ctx_n[:, sub], tidn[:])
                        nc.vector.tensor_copy(ctxT_sb[:], tp[:])
                    ctxT[hp] = ctxT_sb
                while fillers:
                    fillers.pop(0)[0]()
                return ctxT

            # chunk-0 hp0 prerequisites emitted compactly; everything else
            # (hp1 halves, later chunks' units, out_proj) flows as fillers.
            # wave-0 runs e-interleaved across all four units (q0/k0 from
            # the still-idle mm pool, v pairs from pw): consumes each
            # arriving x-piece with ~850ns of PE work, so the startup is
            # DMA-paced with minimal PE idling
            w0q = mmp.tile([128, QC], F32, tag="mm", name="w0q")
            w0k = mmp.tile([128, QC], F32, tag="mm", name="w0k")
            w0v = [pwp.tile([128, 2, EC], F32, tag="pw", name=f"w0v{i}")
                   for i in range(2)]
            for e in range(NE):
                nc.tensor.matmul(w0q[:], twq[:, e, 0:128], txt[e][:, 0:QC],
                                 start=(e == 0), stop=(e == NE - 1))
                nc.tensor.matmul(w0k[:], twk[:, e, 0:128], txt[e][:, 0:QC],
                                 start=(e == 0), stop=(e == NE - 1))
                for i in range(2):
                    for j in range(2):
                        st = 2 * i + j
                        nc.tensor.matmul(
                            w0v[i][:, j, :],
                            txt[e][:, st * 128:(st + 1) * 128],
                            twv[:, e, :],
                            start=(e == 0 and j == 0),
                            stop=(e == NE - 1 and j == 1))
            nc.vector.tensor_scalar_add(tq[0][:, 0:QC], w0q[:], tbq[:, 0:1])
            nc.vector.tensor_scalar_add(tk[0][:, 0:QC], w0k[:], tbk[:, 0:1])
            for i in range(2):
                for j in range(2):
                    st = 2 * i + j
                    nc.vector.tensor_add(
                        tv[:, st, :, :],
                        w0v[i][:, j, :].rearrange("p (h d) -> p h d", h=4),
                        tbv[:].rearrange("p (h d) -> p h d", h=4))
            # chunk processing order: the longest attention (chunk 3,
            # most exp work) runs second-to-last so chunk 2's projection
            # units and out_proj pieces fill its ACT-bound rounds, and
            # the smaller chunk-2 attention absorbs the tail
            order = cfg.get("order", [0, 1, 2, 3])
            emitted = {(0, "q0"), (0, "k0"), (0, "v0"), (0, "v1")}
            pend = []
            ctxT = None
            for idx, qc in enumerate(order):
                last = idx == len(order) - 1
                # fillers: (closure, deadline-round) -- deadlines force
                # emission before the attention rounds that consume them.
                # att(qc) reads k/v of every chunk c <= qc: chunk c's
                # k0/v first feed hp0's k-tile 4c (round 4c+1), its k1
                # feeds hp1's k-tile 4c (round n_kt+1+4c+1), q1 feeds
                # hp1's first round.
                n_kt = 4 * (qc + 1)
                need = []
                for c in range(qc + 1):
                    need += [(c, "k0", 4 * c), (c, "v0", 4 * c + 1),
                             (c, "v1", 4 * c + 1)]
                need += [(qc, "q1", n_kt + 1)]
                for c in range(qc + 1):
                    need += [(c, "k1", n_kt + 1 + 4 * c)]
                need = [(c, w, max(dl, 2)) for c, w, dl in need
                        if (c, w) not in emitted]
                need.sort(key=lambda t: t[2])
                fillers = []
                for c, w, dl in need:
                    fillers += [(p, dl) for p in unit(c, w)]
                    emitted.add((c, w))
                if last:
                    # the final window: pin leftover out_proj pieces to
                    # its late (otherwise ACT-starved) rounds
                    nr = 2 * (n_kt + 1)
                    step = max(1, (nr - n_kt - 2) // max(1, len(pend)))
                    fillers += [(p, n_kt + 2 + i * step)
                                for i, p in enumerate(pend)]
                else:
                    fillers += [(p, 10 ** 9) for p in pend]
                if not last:
                    nxt = order[idx + 1]
                    fillers += [(p, 10 ** 9) for p in unit(nxt, "q0")]
                    emitted.add((nxt, "q0"))
                fillers.sort(key=lambda t: t[1])
                ctxT = attention(qc, fillers, last=last)
                pend = []
                for ss in range(4):
                    pend += op_pieces(qc, ss, ctxT, last=last)
            for fn in pend:
                fn()

    nc.compile()
    return nc


def _get_nc():
    if "nc" not in _CACHE:
        _CACHE["nc"] = _build_nc()
    return _CACHE["nc"]


def make_mask():
    kl = np.arange(128)[:, None]
    ql = np.arange(128)[None, :]
    return (ql >= kl).astype(BF)


def shard_inputs(x, Wq, bq, Wk, bk, Wv, bv, Wo, bo):
    """Build the 8 per-core input maps (host-side sharding)."""
    x = np.asarray(x, dtype=np.float32)
    scale = np.float32(1.0 / np.sqrt(D))
    mask = make_mask()
    ones = np.ones((128, 1), BF)
    idn = np.eye(128, dtype=np.float32).astype(BF)
    in_maps = []
    xTb = [np.ascontiguousarray(np.asarray(x[b]).T).astype(BF)
           for b in range(B)]
    for c in range(NCORES):
        b, g = divmod(c, 4)
        cs = slice(g * EC, (g + 1) * EC)
        in_maps.append({
            "xT": xTb[b],
            "wq": np.ascontiguousarray(
                (np.asarray(Wq[:, cs]) * scale)).astype(BF),
            "wk": np.ascontiguousarray(np.asarray(Wk[:, cs])).astype(BF),
            "wv": np.ascontiguousarray(np.asarray(Wv[:, cs])).astype(BF),
            "wo": np.ascontiguousarray(np.asarray(Wo[cs, :])).astype(BF),
            "bq": (np.asarray(bq[cs]) * scale).reshape(2, 128, 1)
                .astype(np.float32),
            "bk": np.asarray(bk[cs]).reshape(2, 128, 1).astype(np.float32),
            "bv": np.asarray(bv[cs]).reshape(1, EC).astype(np.float32),
            "msk": mask,
            "ones": ones,
            "idn": idn,
        })
    return in_maps


def combine_outputs(results, bo):
    y = np.zeros((B, S, E), np.float32)
    for c in range(NCORES):
        b = c // 4
        y[b] += np.asarray(results[c]["y"], dtype=np.float32)
    y += np.asarray(bo, dtype=np.float32)[None, None, :]
    return y


def kernel(x, Wq, bq, Wk, bk, Wv, bv, Wo, bo):
    from concourse.bass_utils import run_bass_kernel_spmd

    nc = _get_nc()
    in_maps = shard_inputs(x, Wq, bq, Wk, bk, Wv, bv, Wo, bo)
    try:
        res = run_bass_kernel_spmd(nc, in_maps, core_ids=list(range(NCORES)))
    except Exception:
        # transient device errors (e.g. a wedged core) usually clear on retry
        res = run_bass_kernel_spmd(nc, in_maps, core_ids=list(range(NCORES)))
    return combine_outputs(res.results, bo)


# revision 61
# speedup vs baseline: 1.5806x; 1.0048x over previous
"""Causal self-attention (B=2, S=2048, E=1024, H=16) on 8 TRN2 NeuronCores.

Sharding: core c = 4*b + g handles batch b and head-group g (4 heads,
256 E-columns). Each core computes q/k/v projections for its head slice,
causal flash-style attention for its 4 heads, and a partial output
projection y_c = ctx_g @ Wo[rows_g].  Host sums the 4 partials per batch
and adds bo.

All matmul operands are bf16 (inputs converted on host); PSUM accumulation
stays f32.  xT streams in s-chunk-major order interleaved with split
weight loads so projection work starts as soon as the first pieces land.

Attention (q-major context accumulation):
  scoresT [kpos=128, q] per (q-chunk, head-pair, k-tile) -> ACT exp ->
  te bf16 -> gpsimd multiplies a 0/1 triangle into the diagonal block.
  ctx accumulates q-major: out[q=128, d=64] += te-subtile.T @ v-tile
  (te [128,128] stationary, v [128,64] moving -> only 64 rows/matmul);
  the softmax denominator accumulates via an extra 1-row matmul per
  stationary (moving operand = ones column).  All 4 q-subtiles x 2 heads
  of a (q-chunk, head-pair) pack into ONE PSUM bank as a single
  accumulation group; denominators share one bank across the kernel.
  Normalization: per-partition reciprocal [128,4,2] + one DVE multiply
  (denominator broadcast along d via stride-0 AP) -> ctx_n bf16.  PE
  transposes (identity matmul) flip ctx_n to d-major ctxT for the output
  projection.

Scheduling: the attention inner loop is ACT(exp)-throughput-bound, so the
next chunk's projection-wave units and the previous chunk's out_proj
units are emitted as "fillers" between attention (hp, k-tile) rounds --
PE fills its slack with them and every pool stall has independent work
behind it.  Causal trimming: for diagonal k-tile t' only q-columns >=
128*t' are computed (scores, exp, ctx all restricted).
"""

import os

import numpy as np
import ml_dtypes

os.environ.setdefault("NEURON_RT_RESET_CORES", "1")

B, S, E, H, D = 2, 2048, 1024, 16, 64
NCORES = 8
EC = 256          # E-columns per core (4 heads x 64)
QC = 512          # q-chunk width
NQC = S // QC     # 4
NKT = S // 128    # 16 k-tiles
NE = E // 128     # 8 contraction chunks

BF = ml_dtypes.bfloat16

_CACHE = {}


def _build_nc(cfg=None):
    cfg = cfg or {}
    MM_BUFS = cfg.get("mm", 2)
    PW_BUFS = cfg.get("pw", 2)
    EXP_BUFS = cfg.get("exp", 4)
    CTX_BUFS = cfg.get("ctx", 6)
    YP_BUFS = cfg.get("yp", 6)
    import concourse.mybir as mybir
    import concourse.tile as tile
    import concourse.bass as bass
    from concourse import bacc

    F32 = mybir.dt.float32
    BF16 = mybir.dt.bfloat16
    EXP = mybir.ActivationFunctionType.Exp

    nc = bacc.Bacc("TRN2", target_bir_lowering=False, debug=False)

    xT = nc.dram_tensor("xT", [E, S], BF16, kind="ExternalInput")
    wq = nc.dram_tensor("wq", [E, EC], BF16, kind="ExternalInput")
    wk = nc.dram_tensor("wk", [E, EC], BF16, kind="ExternalInput")
    wv = nc.dram_tensor("wv", [E, EC], BF16, kind="ExternalInput")
    wo = nc.dram_tensor("wo", [EC, E], BF16, kind="ExternalInput")
    bq = nc.dram_tensor("bq", [2, 128, 1], F32, kind="ExternalInput")
    bk = nc.dram_tensor("bk", [2, 128, 1], F32, kind="ExternalInput")
    bv = nc.dram_tensor("bv", [1, EC], F32, kind="ExternalInput")
    msk = nc.dram_tensor("msk", [128, 128], BF16, kind="ExternalInput")
    ones = nc.dram_tensor("ones", [128, 1], BF16, kind="ExternalInput")
    idn = nc.dram_tensor("idn", [128, 128], BF16, kind="ExternalInput")

    y = nc.dram_tensor("y", [S, E], BF16, kind="ExternalOutput")

    with tile.TileContext(nc) as tc:
        with (
            tc.tile_pool(name="weights", bufs=1) as wpool,
            tc.tile_pool(name="xtp", bufs=1) as xtp,
            tc.tile_pool(name="qkv", bufs=1) as qkv,
            tc.tile_pool(name="expp", bufs=EXP_BUFS) as expp,
            tc.tile_pool(name="ctxn", bufs=CTX_BUFS) as ctxp,
            tc.tile_pool(name="yp", bufs=YP_BUFS) as yp,
            tc.tile_pool(name="rows", bufs=3) as rows,
            tc.tile_pool(name="smalls", bufs=1) as smalls,
            tc.tile_pool(name="mm", bufs=MM_BUFS, space="PSUM") as mmp,
            tc.tile_pool(name="pw", bufs=PW_BUFS, space="PSUM") as pwp,
            tc.tile_pool(name="cx", bufs=1, space="PSUM") as cxp,
            tc.tile_pool(name="dnp", bufs=1, space="PSUM") as dnp,
        ):
            # ---- small constants (SWDGE/Pool queue; SP stays free) ----
            tbq = smalls.tile([128, 2], F32, tag="bq")
            tbk = smalls.tile([128, 2], F32, tag="bk")
            tbv = smalls.tile([128, EC], F32, tag="bv")
            tmsk = smalls.tile([128, 128], BF16, tag="msk")
            tones = smalls.tile([128, 1], BF16, tag="ones")
            tidn = smalls.tile([128, 128], BF16, tag="idn")

            for r in range(2):
                nc.gpsimd.dma_start(tbq[:, r:r + 1], bq[r])
                nc.gpsimd.dma_start(tbk[:, r:r + 1], bk[r])
            bvap = bv[0, :]
            bv_b = bass.AP(tensor=bvap.tensor, offset=bvap.offset,
                           ap=[[0, 128]] + list(bvap.ap))
            nc.gpsimd.dma_start(tbv[:], bv_b)
            nc.gpsimd.dma_start(tmsk[:], msk[:])
            nc.gpsimd.dma_start(tones[:], ones[:])
            nc.gpsimd.dma_start(tidn[:], idn[:])

            # ---- bulk inputs: s-chunk-major streaming ----
            twq = wpool.tile([128, NE, EC], BF16, tag="wq")
            twk = wpool.tile([128, NE, EC], BF16, tag="wk")
            twv = wpool.tile([128, NE, EC], BF16, tag="wv")
            two = wpool.tile([128, 2, E], BF16, tag="wo")

            def chunked_half(dram, width, h, nch=4):
                a = dram[:]
                return bass.AP(tensor=a.tensor,
                               offset=a.offset + h * nch * 128 * width,
                               ap=[[width, 128], [128 * width, nch],
                                   [1, width]])

            def chunked(dram, nch, width):
                a = dram[:]
                return bass.AP(tensor=a.tensor, offset=a.offset,
                               ap=[[width, 128], [128 * width, nch],
                                   [1, width]])

            txt = [xtp.tile([128, S], BF16, tag=f"xt{e}", name=f"xt{e}")
                   for e in range(NE)]

            def ldx(e, sc, q=None):
                (q or nc.sync).dma_start(
                    txt[e][:, sc * QC:(sc + 1) * QC],
                    xT[e * 128:(e + 1) * 128, sc * QC:(sc + 1) * QC])

            # wave-0 critical path, ordered by first consumer: the
            # e-interleaved wave-0 needs wq/wk/wv A-halves + x(0,0) before
            # its first e-step, then one x piece per step
            nc.sync.dma_start(twq[:, 0:2, :], chunked_half(wq, EC, 0, 2))
            ldx(0, 0, nc.scalar)
            ldx(1, 0, nc.scalar)
            nc.sync.dma_start(
                twq[:, 2:4, :],
                bass.AP(tensor=wq[:].tensor,
                        offset=wq[:].offset + 2 * 128 * EC,
                        ap=[[EC, 128], [128 * EC, 2], [1, EC]]))
            ldx(2, 0, nc.scalar)
            ldx(3, 0, nc.scalar)
            nc.sync.dma_start(twk[:, 0:4, :], chunked_half(wk, EC, 0))
            nc.sync.dma_start(twv[:, 0:4, :], chunked_half(wv, EC, 0))
            nc.sync.dma_start(twq[:, 4:8, :], chunked_half(wq, EC, 1))
            for e in range(4, NE):
                ldx(e, 0)
            nc.sync.dma_start(twk[:, 4:8, :], chunked_half(wk, EC, 1))
            nc.sync.dma_start(twv[:, 4:8, :], chunked_half(wv, EC, 1))
            for e in range(NE):
                ldx(e, 1)
            nc.sync.dma_start(two[:], chunked(wo, 2, E))
            for sc in range(2, NQC):
                for e in range(NE):
                    ldx(e, sc)

            # ---- persistent activation tiles ----
            tq = [qkv.tile([128, S], BF16, tag=f"q{r}", name=f"q{r}")
                  for r in range(2)]
            tk = [qkv.tile([128, S], BF16, tag=f"k{r}", name=f"k{r}")
                  for r in range(2)]
            tv = qkv.tile([128, NKT, 4, 64], BF16, tag="v")

            # broadcast tri-mask [128,128] over the two head-halves
            def mask_b(n):
                m = tmsk[:]
                return bass.AP(tensor=m.tensor, offset=m.offset,
                               ap=[list(m.ap[0]), [0, 2], [1, n]])

            def qk_unit(scn, w, wb, r, tdst):
                """One projection unit: dst[r-half, s-chunk scn] over 8
                contraction steps in one PSUM bank, drained by DVE."""
                sc = slice(scn * QC, (scn + 1) * QC)
                pt = pwp.tile([128, QC], F32, tag="pw",
                              name=f"u{scn}_{r}")
                for e in range(NE):
                    nc.tensor.matmul(
                        pt[:], w[:, e, r * 128:(r + 1) * 128],
                        txt[e][:, sc], start=(e == 0), stop=(e == NE - 1))
                nc.vector.tensor_scalar_add(
                    tdst[r][:, sc], pt[:], wb[:, r:r + 1])

            def v_unit(scn, i):
                v_ps = pwp.tile([128, 2, EC], F32, tag="pw",
                                name=f"pv{scn}_{i}")
                for e in range(NE):
                    for j in range(2):
                        st = 4 * scn + 2 * i + j
                        nc.tensor.matmul(
                            v_ps[:, j, :],
                            txt[e][:, st * 128:(st + 1) * 128],
                            twv[:, e, :],
                            start=(e == 0 and j == 0),
                            stop=(e == NE - 1 and j == 1))
                for j in range(2):
                    st = 4 * scn + 2 * i + j
                    nc.vector.tensor_add(
                        tv[:, st, :, :],
                        v_ps[:, j, :].rearrange("p (h d) -> p h d", h=4),
                        tbv[:].rearrange("p (h d) -> p h d", h=4))

            def qk_pieces(scn, w, wb, r, tdst):
                sc = slice(scn * QC, (scn + 1) * QC)
                cell = {}

                def p0():
                    cell["pt"] = pwp.tile([128, QC], F32, tag="pw",
                                          name=f"u{scn}_{r}")
                    for e in range(4):
                        nc.tensor.matmul(
                            cell["pt"][:], w[:, e, r * 128:(r + 1) * 128],
                            txt[e][:, sc], start=(e == 0), stop=False)

                def p1():
                    pt = cell["pt"]
                    for e in range(4, NE):
                        nc.tensor.matmul(
                            pt[:], w[:, e, r * 128:(r + 1) * 128],
                            txt[e][:, sc], start=False,
                            stop=(e == NE - 1))
                    nc.vector.tensor_scalar_add(
                        tdst[r][:, sc], pt[:], wb[:, r:r + 1])
                return [p0, p1]

            def v_pieces(scn, i):
                cell = {}

                def mk(elo, ehi):
                    def p():
                        if elo == 0:
                            cell["pt"] = pwp.tile(
                                [128, 2, EC], F32, tag="pw",
                                name=f"pv{scn}_{i}")
                        v_ps = cell["pt"]
                        for e in range(elo, ehi):
                            for j in range(2):
                                st = 4 * scn + 2 * i + j
                                nc.tensor.matmul(
                                    v_ps[:, j, :],
                                    txt[e][:, st * 128:(st + 1) * 128],
                                    twv[:, e, :],
                                    start=(e == 0 and j == 0),
                                    stop=(e == NE - 1 and j == 1))
                        if ehi == NE:
                            for j in range(2):
                                st = 4 * scn + 2 * i + j
                                nc.vector.tensor_add(
                                    tv[:, st, :, :],
                                    v_ps[:, j, :].rearrange(
                                        "p (h d) -> p h d", h=4),
                                    tbv[:].rearrange(
                                        "p (h d) -> p h d", h=4))
                    return p
                return [mk(0, 4), mk(4, NE)]

            def unit(scn, which):
                fine = cfg.get("fine", True)
                if which == "q0":
                    ps_ = qk_pieces(scn, twq, tbq, 0, tq)
                elif which == "k0":
                    ps_ = qk_pieces(scn, twk, tbk, 0, tk)
                elif which == "q1":
                    ps_ = qk_pieces(scn, twq, tbq, 1, tq)
                elif which == "k1":
                    ps_ = qk_pieces(scn, twk, tbk, 1, tk)
                elif which == "v0":
                    ps_ = v_pieces(scn, 0)
                else:
                    ps_ = v_pieces(scn, 1)
                if fine:
                    return ps_
                return [lambda: [p() for p in ps_]]

            YENG = cfg.get("yeng", ["dve", "pool", "dve", "pool"])

            tail_dmas = []

            def op_pieces(qc, ss, ctxT, last=False):
                """Output projection for s-subtile ss of chunk qc as two
                filler pieces, each a [128, QC] PSUM half in the pw pool
                (the mm pool stays exclusive to the scores stream)."""
                s0 = qc * QC + ss * 128

                cells = [{}, {}]

                def mk(nn, hp):
                    def p():
                        if hp == 0:
                            # tail chunk: attention is over, so the scores
                            # pool is free -- alternate pw/mm slots to get
                            # more projection pieces in flight
                            tail = last and cfg.get("tailmm", True)
                            pool = mmp if (tail and nn == 0) else pwp
                            tg = "mm" if (tail and nn == 0) else "pw"
                            cells[nn]["py"] = pool.tile(
                                [128, QC], F32, tag=tg,
                                name=f"py{qc}_{ss}_{nn}")
                        py = cells[nn]["py"]
                        nc.tensor.matmul(
                            py[:],
                            ctxT[hp][:, ss, :],
                            two[:, hp, nn * QC:(nn + 1) * QC],
                            start=(hp == 0), stop=(hp == 1))
                        if hp == 0:
                            return
                        ysb = yp.tile([128, QC], BF16, tag="y",
                                      name=f"y{qc}_{ss}_{nn}")
                        # GPSIMD cannot read PSUM on hardware: staging
                        # copies go to DVE, or ACT at the tail (its exp
                        # stream is done by then)
                        if last and (2 * ss + nn) % 2 == 1:
                            nc.scalar.copy(ysb[:], py[:])
                        else:
                            nc.vector.tensor_copy(ysb[:], py[:])
                        if last:
                            # defer the store issue so the ACT queue runs
                            # its staging copies back-to-back first
                            tail_dmas.append(
                                (y[s0:s0 + 128, nn * QC:(nn + 1) * QC],
                                 ysb))
                        else:
                            nc.sync.dma_start(
                                y[s0:s0 + 128, nn * QC:(nn + 1) * QC],
                                ysb[:])
                    return p
                return [lambda a=mk(0, 0), b=mk(0, 1): (a(), b()),
                        lambda a=mk(1, 0), b=mk(1, 1): (a(), b())]

            def attention(qc, fillers, last=False, tail_op=None):
                """fillers: emit-closures (prev chunk's out_proj + next
                chunk's projection units) popped between (hp, kt) rounds
                so PE slack under the ACT-bound exp stream is used."""
                n_kt = 4 * (qc + 1)
                rounds_total = 2 * (n_kt + 1)
                n_fill0 = len([f for f in fillers if f[1] < 2 * 10 ** 9])
                state = {"round": 0, "popped": 0}

                def pop_fillers():
                    state["round"] += 1
                    want = state["round"] * n_fill0 // rounds_total
                    while fillers and (
                            state["popped"] < want
                            or fillers[0][1] <= state["round"]):
                        fillers.pop(0)[0]()
                        state["popped"] += 1

                dn = dnp.tile([128, 2, 4, 2], F32, tag="dn",
                              name=f"dn{qc}")
                ctxT = [None, None]
                for hp in range(2):
                    pctx = cxp.tile([128, 4, 2, 64], F32, tag="cx",
                                    name=f"cx{qc}_{hp}")

                    def ctx_round(kt, te):
                        dg = kt - 4 * qc
                        sub0 = dg if dg > 0 else 0
                        for sub in range(sub0, 4):
                            for h2 in range(2):
                                h = 2 * hp + h2
                                st_ap = te[:, h2 * QC + sub * 128:
                                           h2 * QC + (sub + 1) * 128]
                                first = (kt == 0 and sub == 0 and h2 == 0)
                                last = (kt == n_kt - 1 and sub == 3
                                        and h2 == 1)
                                nc.tensor.matmul(
                                    pctx[:, sub, h2, :], st_ap,
                                    tv[:, kt, h, :],
                                    start=first, stop=last)
                                nc.tensor.matmul(
                                    dn[:, hp, sub, h2:h2 + 1], st_ap,
                                    tones[:],
                                    start=first, stop=last)

                    prev = None
                    for kt in range(n_kt + 1):
                        if kt < n_kt:
                            dg = kt - 4 * qc  # >=0: diagonal tile index
                            coff = 128 * dg if dg > 0 else 0
                            ps = mmp.tile([128, 2 * QC], F32, tag="mm",
                                          name=f"ps{qc}_{hp}_{kt}")
                            te = expp.tile([128, 2 * QC], BF16, tag="exp",
                                           name=f"te{qc}_{hp}_{kt}")
                            for h2 in range(2):
                                bp = h2 * 64
                                nc.tensor.matmul(
                                    ps[:, h2 * QC + coff:(h2 + 1) * QC],
                                    tk[hp][bp:bp + 64,
                                           kt * 128:(kt + 1) * 128],
                                    tq[hp][bp:bp + 64,
                                           qc * QC + coff:(qc + 1) * QC],
                                    start=True, stop=True)
                            if coff:
                                ps3 = ps[:].rearrange("p (t n) -> p t n",
                                                      t=2)
                                te3 = te[:].rearrange("p (t n) -> p t n",
                                                      t=2)
                                nc.scalar.activation(
                                    te3[:, :, coff:QC], ps3[:, :, coff:QC],
                                    EXP)
                            else:
                                nc.scalar.activation(te[:], ps[:], EXP)
                            if dg >= 0:
                                te3 = te[:].rearrange("p (t n) -> p t n",
                                                      t=2)
                                nc.gpsimd.tensor_mul(
                                    te3[:, :, coff:coff + 128],
                                    te3[:, :, coff:coff + 128],
                                    mask_b(128))
                        else:
                            te = None
                        if prev is not None:
                            ctx_round(*prev)
                        prev = (kt, te) if te is not None else None
                        pop_fillers()
                    # ---- normalization + transpose to d-major ----
                    if last and hp == 1:
                        # held-back fillers run here: their PE work covers
                        # the reciprocal+multiply latency before the tail
                        # transposes
                        held = [f for f in fillers
                                if f[1] >= 2 * 10 ** 9]
                        fillers[:] = [f for f in fillers
                                      if f[1] < 2 * 10 ** 9]
                        rec = rows.tile([128, 4, 2], F32, tag="rec")
                        nc.vector.reciprocal(rec[:], dn[:, hp])
                        for fn, _ in held:
                            fn()
                    else:
                        rec = rows.tile([128, 4, 2], F32, tag="rec")
                        nc.vector.reciprocal(rec[:], dn[:, hp])
                    ctx_n = ctxp.tile([128, 4, 2, 64], BF16, tag="ctxn",
                                      name=f"cn{qc}_{hp}")
                    r_ap = rec[:]
                    rec_b = bass.AP(tensor=r_ap.tensor, offset=r_ap.offset,
                                    ap=[list(r_ap.ap[0]), [2, 4], [1, 2],
                                        [0, 64]])
                    ctxT_sb = ctxp.tile([128, 4, 128], BF16, tag="ctxn",
                                        name=f"ct{qc}_{hp}")
                    if not last and not cfg.get("petp", False):
                        nc.vector.tensor_mul(ctx_n[:], pctx[:], rec_b)
                        # SBUF->SBUF crossbar transpose: keeps PE and the
                        # pw PSUM pool out of the normalization chain
                        for sub in range(4):
                            nc.sync.dma_start_transpose(
                                ctxT_sb[:, sub, :], ctx_n[:, sub])
                        ctxT[hp] = ctxT_sb
                    elif hp == 0 or tail_op is None:
                        nc.vector.tensor_mul(ctx_n[:], pctx[:], rec_b)
                        # tail chunk: PE transpose (pw pool is idle by
                        # then, and latency beats the DMA path)
                        tp = pwp.tile([128, 4, 128], BF16, tag="pw",
                                      name=f"tp{qc}_{hp}")
                        for sub in range(4):
                            nc.tensor.transpose(
                                tp[:, sub, :], ctx_n[:, sub], tidn[:])
                        nc.vector.tensor_copy(ctxT_sb[:], tp[:])
                        ctxT[hp] = ctxT_sb
                    else:
                        # final normalization: pipeline per q-subtile so
                        # each out_proj s-tile starts as soon as its own
                        # 128 columns are normalized and transposed
                        tp = pwp.tile([128, 4, 128], BF16, tag="pw",
                                      name=f"tp{qc}_{hp}")
                        ctxT[hp] = ctxT_sb
                        for sub in range(4):
                            nc.vector.tensor_mul(
                                ctx_n[:, sub], pctx[:, sub],
                                bass.AP(tensor=r_ap.tensor,
                                        offset=r_ap.offset + 2 * sub,
                                        ap=[list(r_ap.ap[0]), [1, 2],
                                            [0, 64]]))
                            nc.tensor.transpose(
                                tp[:, sub, :], ctx_n[:, sub], tidn[:])
                            nc.vector.tensor_copy(ctxT_sb[:, sub, :],
                                                  tp[:, sub, :])
                            tail_op(sub, ctxT)
                while fillers:
                    fillers.pop(0)[0]()
                return ctxT

            # chunk-0 hp0 prerequisites emitted compactly; everything else
            # (hp1 halves, later chunks' units, out_proj) flows as fillers.
            # wave-0 runs e-interleaved across all four units (q0/k0 from
            # the still-idle mm pool, v pairs from pw): consumes each
            # arriving x-piece with ~850ns of PE work, so the startup is
            # DMA-paced with minimal PE idling
            nwarm = cfg.get("warm2", 0)
            if nwarm:
                # pace the PE clock ramp during the initial DMA latency:
                # by the time real matmuls arrive the HAM window is warm
                wsb = smalls.tile([128, 256], BF16, tag="warm")
                nc.gpsimd.memset(wsb[:], 0)
                wps = pwp.tile([128, 256], F32, tag="pw", name="warm")
                for i in range(nwarm):
                    nc.tensor.matmul(wps[:], wsb[:, 0:128], wsb[:],
                                     start=True, stop=True)
            w0q = mmp.tile([128, QC], F32, tag="mm", name="w0q")
            w0k = mmp.tile([128, QC], F32, tag="mm", name="w0k")
            w0v = [pwp.tile([128, 2, EC], F32, tag="pw", name=f"w0v{i}")
                   for i in range(2)]
            for e in range(NE):
                nc.tensor.matmul(w0q[:], twq[:, e, 0:128], txt[e][:, 0:QC],
                                 start=(e == 0), stop=(e == NE - 1))
                nc.tensor.matmul(w0k[:], twk[:, e, 0:128], txt[e][:, 0:QC],
                                 start=(e == 0), stop=(e == NE - 1))
                for i in range(2):
                    for j in range(2):
                        st = 2 * i + j
                        nc.tensor.matmul(
                            w0v[i][:, j, :],
                            txt[e][:, st * 128:(st + 1) * 128],
                            twv[:, e, :],
                            start=(e == 0 and j == 0),
                            stop=(e == NE - 1 and j == 1))
            nc.vector.tensor_scalar_add(tq[0][:, 0:QC], w0q[:], tbq[:, 0:1])
            nc.vector.tensor_scalar_add(tk[0][:, 0:QC], w0k[:], tbk[:, 0:1])
            for i in range(2):
                for j in range(2):
                    st = 2 * i + j
                    nc.vector.tensor_add(
                        tv[:, st, :, :],
                        w0v[i][:, j, :].rearrange("p (h d) -> p h d", h=4),
                        tbv[:].rearrange("p (h d) -> p h d", h=4))
            # chunk processing order: the longest attention (chunk 3,
            # most exp work) runs second-to-last so chunk 2's projection
            # units and out_proj pieces fill its ACT-bound rounds, and
            # the smaller chunk-2 attention absorbs the tail
            order = cfg.get("order", [0, 1, 2, 3])
            emitted = {(0, "q0"), (0, "k0"), (0, "v0"), (0, "v1")}
            pend = []
            ctxT = None
            for idx, qc in enumerate(order):
                last = idx == len(order) - 1
                # fillers: (closure, deadline-round) -- deadlines force
                # emission before the attention rounds that consume them.
                # att(qc) reads k/v of every chunk c <= qc: chunk c's
                # k0/v first feed hp0's k-tile 4c (round 4c+1), its k1
                # feeds hp1's k-tile 4c (round n_kt+1+4c+1), q1 feeds
                # hp1's first round.
                n_kt = 4 * (qc + 1)
                need = []
                for c in range(qc + 1):
                    need += [(c, "k0", 4 * c), (c, "v0", 4 * c + 1),
                             (c, "v1", 4 * c + 1)]
                need += [(qc, "q1", n_kt + 1)]
                for c in range(qc + 1):
                    need += [(c, "k1", n_kt + 1 + 4 * c)]
                need = [(c, w, max(dl, 2)) for c, w, dl in need
                        if (c, w) not in emitted]
                need.sort(key=lambda t: t[2])
                fillers = []
                for c, w, dl in need:
                    fillers += [(p, dl) for p in unit(c, w)]
                    emitted.add((c, w))
                if last:
                    # the final window: pin leftover out_proj pieces to
                    # its late (otherwise ACT-starved) rounds
                    nr = 2 * (n_kt + 1)
                    step = max(1, (nr - n_kt - 2) // max(1, len(pend)))
                    nh = cfg.get("hold", 8)
                    fillers += [(p, n_kt + 2 + i * step)
                                if i < len(pend) - nh else (p, 2 * 10 ** 9)
                                for i, p in enumerate(pend)]
                else:
                    fillers += [(p, 10 ** 9) for p in pend]
                if not last:
                    nxt = order[idx + 1]
                    fillers += [(p, 10 ** 9) for p in unit(nxt, "q0")]
                    emitted.add((nxt, "q0"))
                fillers.sort(key=lambda t: t[1])
                if last and cfg.get("subtail", False):
                    def tail_op(ss, ctxT_pair, q=qc):
                        for p in op_pieces(q, ss, ctxT_pair, last=True):
                            p()
                    ctxT = attention(qc, fillers, last=True,
                                     tail_op=tail_op)
                    pend = []
                else:
                    ctxT = attention(qc, fillers, last=last)
                    pend = []
                    for ss in range(4):
                        pend += op_pieces(qc, ss, ctxT, last=last)
            for fn in pend:
                fn()
            for i, (dst, ysb) in enumerate(tail_dmas):
                (nc.sync if i % 2 == 0 else nc.scalar).dma_start(
                    dst, ysb[:])

    nc.compile()
    return nc


def _get_nc():
    if "nc" not in _CACHE:
        _CACHE["nc"] = _build_nc()
    return _CACHE["nc"]


def make_mask():
    kl = np.arange(128)[:, None]
    ql = np.arange(128)[None, :]
    return (ql >= kl).astype(BF)


def shard_inputs(x, Wq, bq, Wk, bk, Wv, bv, Wo, bo):
    """Build the 8 per-core input maps (host-side sharding)."""
    x = np.asarray(x, dtype=np.float32)
    scale = np.float32(1.0 / np.sqrt(D))
    mask = make_mask()
    ones = np.ones((128, 1), BF)
    idn = np.eye(128, dtype=np.float32).astype(BF)
    in_maps = []
    xTb = [np.ascontiguousarray(np.asarray(x[b]).T).astype(BF)
           for b in range(B)]
    for c in range(NCORES):
        b, g = divmod(c, 4)
        cs = slice(g * EC, (g + 1) * EC)
        in_maps.append({
            "xT": xTb[b],
            "wq": np.ascontiguousarray(
                (np.asarray(Wq[:, cs]) * scale)).astype(BF),
            "wk": np.ascontiguousarray(np.asarray(Wk[:, cs])).astype(BF),
            "wv": np.ascontiguousarray(np.asarray(Wv[:, cs])).astype(BF),
            "wo": np.ascontiguousarray(np.asarray(Wo[cs, :])).astype(BF),
            "bq": (np.asarray(bq[cs]) * scale).reshape(2, 128, 1)
                .astype(np.float32),
            "bk": np.asarray(bk[cs]).reshape(2, 128, 1).astype(np.float32),
            "bv": np.asarray(bv[cs]).reshape(1, EC).astype(np.float32),
            "msk": mask,
            "ones": ones,
            "idn": idn,
        })
    return in_maps


def combine_outputs(results, bo):
    y = np.zeros((B, S, E), np.float32)
    for c in range(NCORES):
        b = c // 4
        y[b] += np.asarray(results[c]["y"], dtype=np.float32)
    y += np.asarray(bo, dtype=np.float32)[None, None, :]
    return y


def kernel(x, Wq, bq, Wk, bk, Wv, bv, Wo, bo):
    from concourse.bass_utils import run_bass_kernel_spmd

    nc = _get_nc()
    in_maps = shard_inputs(x, Wq, bq, Wk, bk, Wv, bv, Wo, bo)
    try:
        res = run_bass_kernel_spmd(nc, in_maps, core_ids=list(range(NCORES)))
    except Exception:
        # transient device errors (e.g. a wedged core) usually clear on retry
        res = run_bass_kernel_spmd(nc, in_maps, core_ids=list(range(NCORES)))
    return combine_outputs(res.results, bo)
